# revision 1
# baseline (speedup 1.0000x reference)
"""MBConv (expand 1x1 + BN/ReLU, depthwise 3x3 + BN/ReLU, project 1x1 + BN,
residual) on 8 Trainium2 NeuronCores, data-parallel over the batch.

v2 strategy (vs v1 baseline at 259us):
- All BN folded into conv weights/biases on host; bf16 matmuls.
- Only bf16 I/O: x shipped as bf16, residual added in bf16 (err ~3e-3 ok),
  y returned bf16 and upcast on host.
- conv1 evict: ACT/DVE activation with bias+relu into padded o1.
- depthwise 3x3 split across SIX lanes per (img, ctile) job table:
    "pe"  jobs: 9 diagonal-weight matmuls/chunk accumulating in PSUM
                (cheapest: cost-model matmul = out_free x 0.42ns),
    "mix" jobs: products via DVE tensor_scalar (4x mode, 877ns/pass),
                ACT activation-scale, Pool tensor_scalar; partial sums
                combined with DVE tensor_add (2x) and gpsimd DMA-accumulate
                (SWDGE, rides the mostly-idle DMA engines).
- 5th ctile (64 ch) processed image-PAIRED: two images share one 128-row
  tile so every elementwise pass uses all 128 partitions.
- conv3: residual folded in by preloading PSUM with eye96 @ x16 (one extra
  matmul row-pass), then 5 k-tile matmuls; evict = ACT bias-only.
"""

import sys

for _p in ("/opt/trn_rl_repo", "/root/.axon_site/_ro/trn_rl_repo"):
    if _p not in sys.path:
        sys.path.append(_p)

import ml_dtypes
import numpy as np

import concourse.bass as bass
import concourse.mybir as mybir
import concourse.tile as tile
from concourse import bacc
from concourse.bass_utils import run_bass_kernel_spmd

EPS = 1e-5
N_CORES = 8
NIMG = 4            # images per core
C = 96              # in/out channels
M = 576             # expanded channels
H = W = 56
PIX = H * W         # 3136
WP = 58             # padded width/height
PPIX = WP * WP      # 3364
RCH = 8             # rows per chunk
CHUNK = RCH * W     # 448
NCH = H // RCH      # 7 chunks
CTILES = [(0, 128), (128, 128), (256, 128), (384, 128), (512, 64)]
NCT = len(CTILES)
PE_DG_CTILES = [0, 1, 4]   # ctiles with diag weights staged for PE jobs

F32 = mybir.dt.float32
BF16 = mybir.dt.bfloat16
AOP = mybir.AluOpType
AF = mybir.ActivationFunctionType

# ---- job tables ------------------------------------------------------------
# dw job per (img, ctile<4); ctile 4 runs image-paired (see P5JOB).
#   ("pe", evict_engine)
#   ("mix", nA, nC, nD): nA taps on DVE chain (bias rides first),
#       nC on ACT, nD on Pool, rest (9-nA-nC-nD) DVE products dma-merged;
#       all non-A products accumulate into accA (=o2 tile) via DMA.
PE_EV = {"a": "act", "d": "dve"}
JOB = {
    (0, 0): ("pe", "act"),
    (0, 1): ("pe", "dve"),
    (1, 0): ("pe", "act"),
    (1, 1): ("pe", "dve"),
    (2, 0): ("pe", "act"),
    (3, 0): ("pe", "dve"),
    (2, 1): ("mix", 4, 3, 0),
    (3, 1): ("mix", 4, 3, 0),
    (0, 2): ("mix", 4, 3, 0),
    (0, 3): ("mix", 4, 3, 0),
    (1, 2): ("mix", 4, 3, 0),
    (1, 3): ("mix", 4, 3, 0),
    (2, 2): ("mix", 4, 3, 0),
    (2, 3): ("mix", 4, 3, 0),
    (3, 2): ("mix", 4, 3, 0),
    (3, 3): ("mix", 4, 3, 0),
}
P5JOB = {0: ("pe", "act"), 2: ("pe", "dve")}  # keyed by even img of pair
# conv1 evict engine per (img, ctile): "act" or "dve"
C1EV = {}
for _n in range(NIMG):
    for _m in range(NCT):
        C1EV[(_n, _m)] = "act" if (_n + _m) % 2 == 0 else "dve"

# tap order preference: corners/edges first for chains (any order is valid)
TAPS = list(range(9))

_CACHE = {}


def _build(nimg=NIMG):
    nc = bacc.Bacc("TRN2", target_bir_lowering=False, debug=False)

    # ---- dram tensors -----------------------------------------------------
    x16_d = nc.dram_tensor("x16", [nimg, C, PIX], BF16, kind="ExternalInput").ap()
    w1t_d = nc.dram_tensor("w1t", [C, M], BF16, kind="ExternalInput").ap()
    dg_d = nc.dram_tensor(
        "dg", [128, len(PE_DG_CTILES) * 9 * 128], BF16, kind="ExternalInput"
    ).ap()
    wds_d = nc.dram_tensor("wds", [128, NCT * 9], F32, kind="ExternalInput").ap()
    w3t_d = nc.dram_tensor("w3t", [128, NCT * C], BF16, kind="ExternalInput").ap()
    b1_d = nc.dram_tensor("b1c", [128, NCT], F32, kind="ExternalInput").ap()
    b2_d = nc.dram_tensor("b2c", [128, NCT], F32, kind="ExternalInput").ap()
    b3_d = nc.dram_tensor("b3c", [C, 1], F32, kind="ExternalInput").ap()
    eye_d = nc.dram_tensor("eye96", [C, C], BF16, kind="ExternalInput").ap()
    y_d = nc.dram_tensor("y", [nimg, C, PIX], BF16, kind="ExternalOutput").ap()

    with tile.TileContext(nc) as tc:
        with (
            tc.tile_pool(name="const", bufs=1) as constp,
            tc.tile_pool(name="o1", bufs=1) as o1p,
            tc.tile_pool(name="o2", bufs=1) as o2p,
            tc.tile_pool(name="xin", bufs=1) as xp,
            tc.tile_pool(name="scr", bufs=1) as scrp,
            tc.tile_pool(name="ysb", bufs=2) as yp,
            tc.tile_pool(name="ps1", bufs=2, space="PSUM") as ps1p,
            tc.tile_pool(name="psd", bufs=2, space="PSUM") as psdp,
            tc.tile_pool(name="ps3", bufs=2, space="PSUM") as ps3p,
        ):
            # ---- constants / weights -------------------------------------
            w1t = constp.tile([C, M], BF16)
            nc.sync.dma_start(w1t[:], w1t_d[:])
            b1c = constp.tile([128, NCT], F32)
            nc.sync.dma_start(b1c[:], b1_d[:])
            # first image's input early so conv1(0) can start.
            # 3 rotating buffers: image 3 reuses image 0's (loaded late).
            x16 = []
            for n in range(nimg):
                t = xp.tile([C, PIX], BF16, name=f"x16_{n % 3}", tag=f"x{n % 3}")
                x16.append(t)

            def load_x16(n):
                for q in range(2):
                    nc.sync.dma_start(
                        x16[n][q * 48 : (q + 1) * 48, :],
                        x16_d[n, q * 48 : (q + 1) * 48, :],
                    )

            load_x16(0)
            # PE clock warmup on the small w1t load
            warm = ps1p.tile([C, 1024], F32, name="warm", tag="ps1")
            for _ in range(14):
                nc.tensor.matmul(
                    warm[:, 0:448], w1t[:, 0:C], w1t[:, 0:448],
                    start=True, stop=True,
                )
            dg = constp.tile([128, len(PE_DG_CTILES) * 9 * 128], BF16)
            for i in range(len(PE_DG_CTILES)):
                nc.sync.dma_start(
                    dg[:, i * 9 * 128 : (i + 1) * 9 * 128],
                    dg_d[:, i * 9 * 128 : (i + 1) * 9 * 128],
                )
            wds = constp.tile([128, NCT * 9], F32)
            nc.sync.dma_start(wds[:], wds_d[:])
            w3t = constp.tile([128, NCT * C], BF16)
            nc.sync.dma_start(w3t[:], w3t_d[:])
            b2c = constp.tile([128, NCT], F32)
            nc.sync.dma_start(b2c[:], b2_d[:])
            b3c = constp.tile([C, 1], F32)
            nc.sync.dma_start(b3c[:], b3_d[:])
            eye96 = constp.tile([C, C], BF16)
            nc.sync.dma_start(eye96[:], eye_d[:])
            for n in range(1, 3):
                load_x16(n)

            # ---- persistent activation buffers ---------------------------
            # o1: padded conv1 outputs. ctiles 0-3: slot = n % 2.
            # ctile 4: image-paired, slot = (n // 2) % 2.
            o1 = {}
            o2 = {}
            for m in range(NCT):
                for s in range(2):
                    t1 = o1p.tile([128, PPIX], BF16, name=f"o1_{m}_{s}")
                    t1r = t1.rearrange("p (r c) -> p r c", c=WP)
                    nc.gpsimd.memset(t1[:, 0:WP], 0.0)
                    nc.gpsimd.memset(t1[:, PPIX - WP : PPIX], 0.0)
                    nc.gpsimd.memset(t1r[:, :, 0:1], 0.0)
                    nc.gpsimd.memset(t1r[:, :, WP - 1 : WP], 0.0)
                    o1[(m, s)] = t1
                    o2[(m, s)] = o2p.tile([128, PIX], BF16, name=f"o2_{m}_{s}")

            def slot(n, m):
                return (n // 2) % 2 if m == 4 else n % 2

            # ------------------------------------------------------------------
            # stage emitters
            # ------------------------------------------------------------------
            def conv1(n, m):
                """conv1 ctile m of image n (ctile 4: n and n+1 paired)."""
                cs, P = CTILES[m]
                s = slot(n, m)
                o1r = o1[(m, s)].rearrange("p (r c) -> p r c", c=WP)
                for j0 in range(0, NCH, 2):
                    npair = min(2, NCH - j0)
                    ps = ps1p.tile([128, 1024], F32, name="ps1", tag="ps1")
                    for g in range(npair):
                        c0 = (j0 + g) * CHUNK
                        if m == 4:
                            nc.tensor.matmul(
                                ps[0:64, g * 512 : g * 512 + CHUNK],
                                w1t[:, cs : cs + 64],
                                x16[n][:, c0 : c0 + CHUNK],
                                start=True, stop=True,
                            )
                            nc.tensor.matmul(
                                ps[64:128, g * 512 : g * 512 + CHUNK],
                                w1t[:, cs : cs + 64],
                                x16[n + 1][:, c0 : c0 + CHUNK],
                                start=True, stop=True,
                            )
                        else:
                            nc.tensor.matmul(
                                ps[0:P, g * 512 : g * 512 + CHUNK],
                                w1t[:, cs : cs + P],
                                x16[n][:, c0 : c0 + CHUNK],
                                start=True, stop=True,
                            )
                    PP = 128 if m == 4 else P
                    if npair == 2:
                        src = (
                            ps.rearrange("p (g x) -> p g x", g=2)[0:PP, :, 0:CHUNK]
                            .rearrange("p g (r c) -> p g r c", c=56)
                        )
                        dst = o1r[
                            0:PP, j0 * RCH + 1 : j0 * RCH + 2 * RCH + 1, 1:57
                        ].rearrange("p (g r) c -> p g r c", g=2)
                    else:
                        src = ps[0:PP, 0:CHUNK]
                        dst = o1r[0:PP, j0 * RCH + 1 : j0 * RCH + RCH + 1, 1:57]
                    if C1EV[(n, m)] == "act":
                        nc.scalar.activation(
                            dst, src, AF.Relu, bias=b1c[0:PP, m : m + 1]
                        )
                    else:
                        nc.vector.tensor_scalar(
                            dst, src, b1c[0:PP, m : m + 1], 0.0, AOP.add, AOP.max
                        )

            def dw_pe(n, m, ev):
                """depthwise via 9 diag matmuls/chunk on PE (ctile 4: paired,
                full 128 partitions via duplicated diag blocks)."""
                cs, P = CTILES[m]
                PP = 128 if m == 4 else P
                s = slot(n, m)
                o1r = o1[(m, s)].rearrange("p (r c) -> p r c", c=WP)
                o2t = o2[(m, s)]
                di = PE_DG_CTILES.index(m)
                for j in range(NCH):
                    ps = psdp.tile([128, CHUNK], F32, name="psd", tag="psd")
                    for k in range(9):
                        ky, kx = divmod(k, 3)
                        nc.tensor.matmul(
                            ps[0:PP, :],
                            dg[0:PP, (di * 9 + k) * 128 : (di * 9 + k) * 128 + PP],
                            o1r[0:PP, j * RCH + ky : j * RCH + ky + 8, kx : kx + 56],
                            start=(k == 0),
                            stop=(k == 8),
                        )
                    dst2 = o2t[0:PP, j * CHUNK : (j + 1) * CHUNK]
                    if ev == "act":
                        nc.scalar.activation(
                            dst2, ps[0:PP, :], AF.Relu, bias=b2c[0:PP, m : m + 1]
                        )
                    else:
                        nc.vector.tensor_scalar(
                            dst2, ps[0:PP, :], b2c[0:PP, m : m + 1], 0.0,
                            AOP.add, AOP.max,
                        )

            def dw_mix(n, m, nA, nC, nD):
                """depthwise via DVE/ACT/Pool products + DVE/DMA adds.

                accA = o2 tile accumulates: A-chain on DVE (tt_add), all other
                products merged in via gpsimd DMA-accumulate.
                """
                cs, P = CTILES[m]
                s = slot(n, m)
                o1r = o1[(m, s)].rearrange("p (r c) -> p r c", c=WP)
                acc = o2[(m, s)]
                PP = 128 if m == 4 else P

                def tap(k):
                    ky, kx = divmod(k, 3)
                    return o1r[0:PP, ky : ky + 56, kx : kx + 56]

                wd1 = lambda k: wds[0:PP, m * 9 + k : m * 9 + k + 1]

                ks = TAPS
                A = ks[:nA]
                Cc = ks[nA : nA + nC]
                D = ks[nA + nC : nA + nC + nD]
                B = ks[nA + nC + nD :]

                # all 9 products first (each reads o1; emitting them
                # back-to-back releases the o1 slot as early as possible),
                # then the adds. Bias rides the acc-init product.
                nc.vector.tensor_scalar(
                    acc[0:PP, :], tap(A[0]), wd1(A[0]), b2c[0:PP, m : m + 1],
                    AOP.mult, AOP.add,
                )
                apods = []
                for k in A[1:]:
                    q = scrp.tile([128, PIX], BF16, name="pb", tag="pb", bufs=3)
                    nc.vector.tensor_scalar(
                        q[0:PP, :], tap(k), wd1(k), None, AOP.mult
                    )
                    apods.append(q)
                merge = []  # tiles to DMA-accumulate into acc
                for k in B:
                    q = scrp.tile([128, PIX], BF16, name="qb", tag="qb", bufs=2)
                    nc.vector.tensor_scalar(
                        q[0:PP, :], tap(k), wd1(k), None, AOP.mult
                    )
                    merge.append(q)
                for k in Cc:
                    q = scrp.tile([128, PIX], BF16, name="qc", tag="qc", bufs=3)
                    nc.scalar.activation(
                        q[0:PP, :], tap(k), AF.Copy, scale=wd1(k)
                    )
                    merge.append(q)
                for k in D:
                    q = scrp.tile([128, PIX], BF16, name="qd", tag="qd", bufs=1)
                    nc.gpsimd.tensor_scalar(
                        q[0:PP, :], tap(k), wd1(k), None, AOP.mult
                    )
                    merge.append(q)
                for q in apods:
                    nc.vector.tensor_add(acc[0:PP, :], acc[0:PP, :], q[0:PP, :])
                # DMA-accumulate. NOTE: accum DMAs corrupt when the AP needs
                # descriptor splitting (inner run > 2048 elems) — issue as two
                # half-column 2D DMAs. Pair-merge first to shorten the tail
                # of serial accumulates on acc.
                HF = PIX // 2

                def dma_acc(dst, src):
                    for h0, h1 in ((0, HF), (HF, PIX)):
                        nc.gpsimd.dma_start(
                            dst[0:PP, h0:h1], src[0:PP, h0:h1], accum_op=AOP.add
                        )

                # tree-reduce merge tiles pairwise, then a single
                # accumulate into acc keeps the acc dependency tail short
                while len(merge) > 1:
                    nxt = []
                    for i in range(0, len(merge) - 1, 2):
                        dma_acc(merge[i], merge[i + 1])
                        nxt.append(merge[i])
                    if len(merge) % 2 == 1:
                        nxt.append(merge[-1])
                    merge = nxt
                if merge:
                    dma_acc(acc, merge[0])
                # final relu in place
                nc.vector.tensor_scalar_max(acc[0:PP, :], acc[0:PP, :], 0.0)

            def dw(n):
                jobs = [(m, JOB[(n, m)]) for m in range(4)]
                if n % 2 == 1:  # paired ctile-4 job after conv1 of both images
                    jobs.append((4, P5JOB[n - 1]))
                # mix jobs first: their DMA tails overlap the pe matmuls
                for m, jb in jobs:
                    if jb[0] == "mix":
                        dw_mix(n if m < 4 else n - 1, m, jb[1], jb[2], jb[3])
                for m, jb in jobs:
                    if jb[0] == "pe":
                        dw_pe(n if m < 4 else n - 1, m, jb[1])

            def conv3(n):
                half = 0 if n % 2 == 0 else 64
                for j in range(NCH):
                    ps = ps3p.tile([128, CHUNK], F32, name="ps3", tag="ps3")
                    # residual preload: ps = eye96 @ x16 chunk
                    nc.tensor.matmul(
                        ps[0:C, :],
                        eye96[:, :],
                        x16[n][:, j * CHUNK : (j + 1) * CHUNK],
                        start=True, stop=False,
                    )
                    for kt in range(4):
                        nc.tensor.matmul(
                            ps[0:C, :],
                            w3t[0:128, kt * C : kt * C + C],
                            o2[(kt, n % 2)][0:128, j * CHUNK : (j + 1) * CHUNK],
                            start=False, stop=False,
                        )
                    nc.tensor.matmul(
                        ps[0:C, :],
                        w3t[half : half + 64, 4 * C : 4 * C + C],
                        o2[(4, (n // 2) % 2)][
                            half : half + 64, j * CHUNK : (j + 1) * CHUNK
                        ],
                        start=False, stop=True,
                    )
                    o3 = yp.tile([C, CHUNK], BF16, name="o3", tag="o3")
                    nc.scalar.activation(
                        o3[:], ps[0:C, :], AF.Identity, bias=b3c[:]
                    )
                    nc.sync.dma_start(
                        y_d[n, :, j * CHUNK : (j + 1) * CHUNK], o3[:]
                    )

            # ---- pipeline ----------------------------------------------------
            # c1(0) c1(1) dw(0) dw(1)+p5(01) c3(0) c1(2) dw(2) c3(1)
            # c1(3) dw(3)+p5(23) c3(2) c3(3)
            for m in range(NCT):
                conv1(0, m)
                if m < 4:
                    conv1(1, m)
            dw(0)
            for m in range(NCT):
                conv1(2, m)
            dw(1)
            conv3(0)
            load_x16(3)
            for m in range(4):
                conv1(3, m)
            dw(2)
            conv3(1)
            dw(3)
            conv3(2)
            conv3(3)

    nc.compile()
    return nc


def _fold_bn(inputs):
    """fold BN params into conv weights/biases; build device-side arrays"""
    f = lambda k: np.asarray(inputs[k], np.float32)
    w1, g1, b1, m1, v1 = f("w1"), f("g1"), f("b1"), f("m1"), f("v1")
    wd, g2, b2, m2, v2 = f("wd"), f("g2"), f("b2"), f("m2"), f("v2")
    w3, g3, b3, m3, v3 = f("w3"), f("g3"), f("b3"), f("m3"), f("v3")

    s1 = g1 / np.sqrt(v1 + EPS)
    W1p = w1[:, :, 0, 0] * s1[:, None]              # [M, C]
    b1p = b1 - m1 * s1                              # [M]
    s2 = g2 / np.sqrt(v2 + EPS)
    wdp = wd[:, 0] * s2[:, None, None]              # [M, 3, 3]
    b2p = b2 - m2 * s2
    s3 = g3 / np.sqrt(v3 + EPS)
    W3p = w3[:, :, 0, 0] * s3[:, None]              # [C, M]
    b3p = b3 - m3 * s3

    w1t = np.ascontiguousarray(W1p.T).astype(ml_dtypes.bfloat16)  # [C, M]

    dgm = np.zeros((128, len(PE_DG_CTILES) * 9 * 128), np.float32)
    wds = np.zeros((128, NCT * 9), np.float32)
    for m, (cs, P) in enumerate(CTILES):
        for k in range(9):
            ky, kx = divmod(k, 3)
            w = wdp[cs : cs + P, ky, kx]
            if m == 4:
                wds[0:64, m * 9 + k] = w
                wds[64:128, m * 9 + k] = w
            else:
                wds[:P, m * 9 + k] = w
            if m in PE_DG_CTILES:
                di = PE_DG_CTILES.index(m)
                if m == 4:
                    blk = dgm[:128, (di * 9 + k) * 128 : (di * 9 + k) * 128 + 128]
                    np.fill_diagonal(blk, np.concatenate([w, w]))
                else:
                    blk = dgm[:P, (di * 9 + k) * 128 : (di * 9 + k) * 128 + P]
                    np.fill_diagonal(blk, w)
    dgm = dgm.astype(ml_dtypes.bfloat16)

    w3t = np.zeros((128, NCT * C), np.float32)
    for kt, (ks, K) in enumerate(CTILES):
        w3t[:K, kt * C : kt * C + C] = W3p.T[ks : ks + K, :]
        if kt == 4:
            w3t[64:128, kt * C : kt * C + C] = W3p.T[ks : ks + K, :]
    w3t = w3t.astype(ml_dtypes.bfloat16)

    b1c = np.zeros((128, NCT), np.float32)
    b2c = np.zeros((128, NCT), np.float32)
    for m, (cs, P) in enumerate(CTILES):
        b1c[:P, m] = b1p[cs : cs + P]
        b2c[:P, m] = b2p[cs : cs + P]
        if m == 4:
            b1c[64:128, m] = b1p[cs : cs + P]
            b2c[64:128, m] = b2p[cs : cs + P]
    b3c = b3p.reshape(C, 1).astype(np.float32)
    eye = np.eye(C, dtype=np.float32).astype(ml_dtypes.bfloat16)

    return dict(
        w1t=w1t, dg=dgm, wds=wds, w3t=w3t, b1c=b1c, b2c=b2c, b3c=b3c, eye96=eye
    )


def _in_maps(inputs):
    params = _fold_bn(inputs)
    x = np.asarray(inputs["x"], np.float32)
    xr = x.reshape(N_CORES, NIMG, C, PIX)
    return [
        dict(
            x16=np.ascontiguousarray(xr[c]).astype(ml_dtypes.bfloat16),
            **params,
        )
        for c in range(N_CORES)
    ]


def kernel(**inputs):
    if "nc" not in _CACHE:
        _CACHE["nc"] = _build()
    nc = _CACHE["nc"]

    B = np.asarray(inputs["x"]).shape[0]
    in_maps = _in_maps(inputs)
    res = run_bass_kernel_spmd(nc, in_maps, core_ids=list(range(N_CORES)))
    out = np.stack(
        [np.asarray(res.results[c]["y"]).astype(np.float32) for c in range(N_CORES)]
    )
    return out.reshape(B, C, H, W)


if __name__ == "__main__":
    rng = np.random.default_rng(0)
    inputs = dict(
        x=rng.standard_normal((32, C, H, W), dtype=np.float32),
        w1=(rng.standard_normal((M, C, 1, 1)) * 0.05).astype(np.float32),
        g1=np.ones(M, np.float32), b1=np.zeros(M, np.float32),
        m1=(rng.standard_normal(M) * 0.1).astype(np.float32),
        v1=rng.uniform(0.5, 1.5, M).astype(np.float32),
        wd=(rng.standard_normal((M, 1, 3, 3)) * 0.1).astype(np.float32),
        g2=np.ones(M, np.float32), b2=np.zeros(M, np.float32),
        m2=(rng.standard_normal(M) * 0.1).astype(np.float32),
        v2=rng.uniform(0.5, 1.5, M).astype(np.float32),
        w3=(rng.standard_normal((C, M, 1, 1)) * 0.05).astype(np.float32),
        g3=np.ones(C, np.float32), b3=np.zeros(C, np.float32),
        m3=np.zeros(C, np.float32), v3=np.ones(C, np.float32),
    )
    out = kernel(**inputs)
    print("kernel out", out.shape, out.dtype)



# revision 43
# speedup vs baseline: 1.3730x; 1.3730x over previous
"""MBConv (expand 1x1 + BN/ReLU, depthwise 3x3 + BN/ReLU, project 1x1 + BN,
residual) on 8 Trainium2 NeuronCores, data-parallel over the batch.

v3 strategy (vs v2 at 261.5us):
- f16 activations/weights everywhere (same cost-model speed as bf16 in every
  path: matmul 1 cyc/row, DVE 4x/2x modes, ACT/Pool/DMA unchanged) for ~8x
  better mantissa -> headroom under the 2e-2 gate.
- conv1 (PE) with ALL evictions on ACT (fused relu+bias, 2-chunk groups).
- dw split by tile route, balanced via LP against the TimelineSim cost model:
    * PE route (7 tiles): 9 diag-matmuls/chunk accumulating in PSUM
      (188ns/chunk-tap at full clock), ACT evict relu+bias.
    * DVE route (11 tiles): tap products via DVE tensor_scalar (877ns/pass,
      4x mode) -- bias rides the first product; 3 products merged with DVE
      tensor_tensor adds (1694ns); 4-5 products merged via gpsimd
      DMA-accumulate tree (Pool desc-gen ~1us + DMA ~1.1us per half);
      1-2 products per tile on ACT (2798ns); final relu on DVE.
- conv3 (PE): residual folded via eye96 @ x16 PSUM preload; eviction + bias
  on Pool (gpsimd tensor_scalar, 717ns/chunk).
- Engine budgets ~PE 140 / DVE 137 / ACT 138 / Pool 138 / DMA 134 us.
"""

import sys

for _p in ("/opt/trn_rl_repo", "/root/.axon_site/_ro/trn_rl_repo"):
    if _p not in sys.path:
        sys.path.append(_p)

import numpy as np

import concourse.bass as bass
import concourse.mybir as mybir
import concourse.tile as tile
from concourse import bacc
from concourse.bass_utils import run_bass_kernel_spmd

EPS = 1e-5
N_CORES = 8
NIMG = 4            # images per core
C = 96              # in/out channels
M = 576             # expanded channels
H = W = 56
PIX = H * W         # 3136
WP = 58             # padded width/height
PPIX = WP * WP      # 3364
RCH = 8             # rows per chunk
CHUNK = RCH * W     # 448
NCH = H // RCH      # 7 chunks
HF = PIX // 2       # half-image cols for accum DMAs (inner run <= 2048)
CTILES = [(0, 128), (128, 128), (256, 128), (384, 128), (512, 64)]
NCT = len(CTILES)

F32 = mybir.dt.float32
F16 = mybir.dt.float16
AOP = mybir.AluOpType
AF = mybir.ActivationFunctionType

# ---- route tables ----------------------------------------------------------
# dw tiles are keyed (img, ctile) for ctile<4 and (pair_img, 4) with
# pair_img in {0, 2} for the image-paired 64-channel ctile.
# PE-routed tiles need diag weights staged; biased toward later images so
# DVE-route work front-loads and drains before the final conv3s.
PE_TILES = {(0, 0), (0, 1), (0, 4), (1, 0), (1, 1), (2, 0), (2, 4), (3, 0)}
PE_DG_CTILES = sorted({m for (_n, m) in PE_TILES})
# tiles whose tap 8 also goes to ACT (2 ACT products instead of 1)
ACT2_TILES = {(2, 2), (2, 3), (3, 1), (3, 2), (3, 3)}
# tiles whose tap 7 goes to DVE instead of ACT (phase-0 ACT relief)
DVE7_TILES = set()
# tiles merged entirely on DVE/ACT (TT adds, no DMA) -- for the tail, where
# DVE is idle and the DMA-accum latency chain would gate the last conv3
TT_TILES = {(3, 3)}
HROWS = 28          # rows per G-tile half (dw processed in row-halves)

_CACHE = {}


def _build(nimg=NIMG):
    nc = bacc.Bacc("TRN2", target_bir_lowering=False, debug=False)

    # ---- dram tensors -----------------------------------------------------
    x16_d = nc.dram_tensor("x16", [nimg, C, PIX], F16, kind="ExternalInput").ap()
    w1t_d = nc.dram_tensor("w1t", [C, M], F16, kind="ExternalInput").ap()
    dg_d = nc.dram_tensor(
        "dg", [128, len(PE_DG_CTILES) * 9 * 128], F16, kind="ExternalInput"
    ).ap()
    wds_d = nc.dram_tensor("wds", [128, NCT * 9], F32, kind="ExternalInput").ap()
    w3t_d = nc.dram_tensor("w3t", [128, NCT * C], F16, kind="ExternalInput").ap()
    b1_d = nc.dram_tensor("b1c", [128, NCT], F32, kind="ExternalInput").ap()
    b2_d = nc.dram_tensor("b2c", [128, NCT], F32, kind="ExternalInput").ap()
    b3_d = nc.dram_tensor("b3c", [C, 1], F32, kind="ExternalInput").ap()
    eye_d = nc.dram_tensor("eye96", [C, C], F16, kind="ExternalInput").ap()
    y_d = nc.dram_tensor("y", [nimg, C, PIX], F16, kind="ExternalOutput").ap()

    with tile.TileContext(nc) as tc:
        with (
            tc.tile_pool(name="const", bufs=1) as constp,
            tc.tile_pool(name="o1", bufs=1) as o1p,
            tc.tile_pool(name="o2", bufs=1) as o2p,
            tc.tile_pool(name="xin", bufs=1) as xp,
            tc.tile_pool(name="scr", bufs=1) as scrp,
            tc.tile_pool(name="ysb", bufs=2) as yp,
            tc.tile_pool(name="ps1", bufs=2, space="PSUM") as ps1p,
            tc.tile_pool(name="psd", bufs=2, space="PSUM") as psdp,
            tc.tile_pool(name="ps3", bufs=2, space="PSUM") as ps3p,
        ):
            # ---- constants / weights -------------------------------------
            w1t = constp.tile([C, M], F16)
            nc.sync.dma_start(w1t[:], w1t_d[:])
            b1c = constp.tile([128, NCT], F32)
            nc.sync.dma_start(b1c[:], b1_d[:])
            # x buffers: 3 rotating, image 3 reuses image 0's slot.
            x16 = []
            for n in range(nimg):
                t = xp.tile([C, PIX], F16, name=f"x16_{n % 3}", tag=f"x{n % 3}")
                x16.append(t)

            def load_x16(n):
                # column-split: first half unblocks conv1 chunks 0-3 early
                for q in range(2):
                    nc.sync.dma_start(
                        x16[n][:, q * HF : (q + 1) * HF],
                        x16_d[n, :, q * HF : (q + 1) * HF],
                    )

            load_x16(0)
            # PE clock warmup while weights load (ramp to full pstate)
            warm = ps1p.tile([C, 1024], F32, name="warm", tag="ps1")
            for _ in range(10):
                nc.tensor.matmul(
                    warm[:, 0:448], w1t[:, 0:C], w1t[:, 0:448],
                    start=True, stop=True,
                )
            dg = constp.tile([128, len(PE_DG_CTILES) * 9 * 128], F16)
            for i in range(len(PE_DG_CTILES)):
                nc.sync.dma_start(
                    dg[:, i * 9 * 128 : (i + 1) * 9 * 128],
                    dg_d[:, i * 9 * 128 : (i + 1) * 9 * 128],
                )
            wds = constp.tile([128, NCT * 9], F32)
            nc.sync.dma_start(wds[:], wds_d[:])
            w3t = constp.tile([128, NCT * C], F16)
            nc.sync.dma_start(w3t[:], w3t_d[:])
            b2c = constp.tile([128, NCT], F32)
            nc.sync.dma_start(b2c[:], b2_d[:])
            b3c = constp.tile([C, 1], F32)
            nc.sync.dma_start(b3c[:], b3_d[:])
            eye96 = constp.tile([C, C], F16)
            nc.sync.dma_start(eye96[:], eye_d[:])
            for n in range(1, 3):
                load_x16(n)

            # ---- persistent activation buffers ---------------------------
            # o1: padded conv1 outputs. ctiles 0-3: slot = n % 2.
            # ctile 4: image-paired, slot = (n // 2) % 2.
            o1 = {}
            o2 = {}
            for m in range(NCT):
                for s in range(2):
                    t1 = o1p.tile([128, PPIX], F16, name=f"o1_{m}_{s}")
                    t1r = t1.rearrange("p (r c) -> p r c", c=WP)
                    nc.gpsimd.memset(t1[:, 0:WP], 0.0)
                    nc.gpsimd.memset(t1[:, PPIX - WP : PPIX], 0.0)
                    nc.gpsimd.memset(t1r[:, :, 0:1], 0.0)
                    nc.gpsimd.memset(t1r[:, :, WP - 1 : WP], 0.0)
                    o1[(m, s)] = t1
                    o2[(m, s)] = o2p.tile([128, PIX], F16, name=f"o2_{m}_{s}")

            def slot(n, m):
                return (n // 2) % 2 if m == 4 else n % 2

            # ------------------------------------------------------------------
            # stage emitters
            # ------------------------------------------------------------------
            def conv1(n, m):
                """conv1 ctile m of image n (ctile 4: n and n+1 paired).
                Evictions on ACT (fused relu + bias); images 0-1 alternate
                ACT/Pool per group (early ACT is eviction-throughput-bound)."""
                cs, P = CTILES[m]
                s = slot(n, m)
                o1r = o1[(m, s)].rearrange("p (r c) -> p r c", c=WP)
                for j0 in range(0, NCH, 2):
                    npair = min(2, NCH - j0)
                    ps = ps1p.tile([128, 1024], F32, name="ps1", tag="ps1")
                    for g in range(npair):
                        c0 = (j0 + g) * CHUNK
                        if m == 4:
                            nc.tensor.matmul(
                                ps[0:64, g * 512 : g * 512 + CHUNK],
                                w1t[:, cs : cs + 64],
                                x16[n][:, c0 : c0 + CHUNK],
                                start=True, stop=True,
                            )
                            nc.tensor.matmul(
                                ps[64:128, g * 512 : g * 512 + CHUNK],
                                w1t[:, cs : cs + 64],
                                x16[n + 1][:, c0 : c0 + CHUNK],
                                start=True, stop=True,
                            )
                        else:
                            nc.tensor.matmul(
                                ps[0:P, g * 512 : g * 512 + CHUNK],
                                w1t[:, cs : cs + P],
                                x16[n][:, c0 : c0 + CHUNK],
                                start=True, stop=True,
                            )
                    PP = 128 if m == 4 else P
                    if npair == 2:
                        src = (
                            ps.rearrange("p (g x) -> p g x", g=2)[0:PP, :, 0:CHUNK]
                            .rearrange("p g (r c) -> p g r c", c=56)
                        )
                        dst = o1r[
                            0:PP, j0 * RCH + 1 : j0 * RCH + 2 * RCH + 1, 1:57
                        ].rearrange("p (g r) c -> p g r c", g=2)
                    else:
                        src = ps[0:PP, 0:CHUNK]
                        dst = o1r[0:PP, j0 * RCH + 1 : j0 * RCH + RCH + 1, 1:57]
                    nc.scalar.activation(
                        dst, src, AF.Relu, bias=b1c[0:PP, m : m + 1]
                    )

            def dw_pe(n, m):
                """depthwise via 9 diag matmuls/chunk on PE (ctile 4: paired,
                full 128 partitions via duplicated diag blocks); ACT evict."""
                cs, P = CTILES[m]
                PP = 128 if m == 4 else P
                s = slot(n, m)
                o1r = o1[(m, s)].rearrange("p (r c) -> p r c", c=WP)
                o2t = o2[(m, s)]
                di = PE_DG_CTILES.index(m)
                for j in range(NCH):
                    ps = psdp.tile([128, CHUNK], F32, name="psd", tag="psd")
                    for k in range(9):
                        ky, kx = divmod(k, 3)
                        nc.tensor.matmul(
                            ps[0:PP, :],
                            dg[0:PP, (di * 9 + k) * 128 : (di * 9 + k) * 128 + PP],
                            o1r[0:PP, j * RCH + ky : j * RCH + ky + 8, kx : kx + 56],
                            start=(k == 0),
                            stop=(k == 8),
                        )
                    nc.scalar.activation(
                        o2t[0:PP, j * CHUNK : (j + 1) * CHUNK],
                        ps[0:PP, :], AF.Relu, bias=b2c[0:PP, m : m + 1],
                    )

            def dw_g_half(n, m, h):
                """one row-half of a DVE-route depthwise tile.

                All products and merges land in scratch; o2 is written only by
                the final relu pass (keeps the write-after-read window vs
                conv3 of image n-2 to a single 468ns op). Merge: 3 DVE
                tensor_tensor adds into q0, 5 gpsimd DMA-accumulates
                (single-DMA per add: inner run 1568 elems < 2048 so no
                descriptor splitting corruption).
                """
                cs, P = CTILES[m]
                s = slot(n, m)
                o1r = o1[(m, s)].rearrange("p (r c) -> p r c", c=WP)
                acc = o2[(m, s)][:, h * HROWS * W : (h + 1) * HROWS * W]
                PP = 128 if m == 4 else P
                two_act = (n, m) in ACT2_TILES

                def tap(k):
                    ky, kx = divmod(k, 3)
                    return o1r[
                        0:PP, h * HROWS + ky : h * HROWS + ky + HROWS, kx : kx + 56
                    ]

                wd1 = lambda k: wds[0:PP, m * 9 + k : m * 9 + k + 1]
                HP = HROWS * W

                qt = {}

                def q(k, tag, bufs):
                    qt[k] = scrp.tile([128, HP], F16, name=tag, tag=tag, bufs=bufs)
                    return qt[k]

                # products (back-to-back to free the o1 slot early)
                nc.vector.tensor_scalar(
                    q(0, "qe", 4)[0:PP, :], tap(0), wd1(0), b2c[0:PP, m : m + 1],
                    AOP.mult, AOP.add,
                )
                for k in (1, 2, 3):   # DVE -> TT-added into q0
                    nc.vector.tensor_scalar(
                        q(k, "qa", 2)[0:PP, :], tap(k), wd1(k), None, AOP.mult
                    )
                tt_tile = (n, m) in TT_TILES
                for k in (4, 5, 6):   # DVE -> DMA-merged (or TT for TT_TILES)
                    nc.vector.tensor_scalar(
                        q(k, "qb", 5)[0:PP, :], tap(k), wd1(k), None, AOP.mult
                    )
                # ACT products (tap 7 unless DVE7; tap 8 if two_act else DVE)
                if (n, m) in DVE7_TILES:
                    nc.vector.tensor_scalar(
                        q(7, "qc", 2)[0:PP, :], tap(7), wd1(7), None, AOP.mult
                    )
                else:
                    nc.scalar.activation(
                        q(7, "qc", 2)[0:PP, :], tap(7), AF.Copy, scale=wd1(7)
                    )
                if two_act:
                    nc.scalar.activation(
                        q(8, "qd", 3)[0:PP, :], tap(8), AF.Copy, scale=wd1(8)
                    )
                else:
                    nc.vector.tensor_scalar(
                        q(8, "qd", 3)[0:PP, :], tap(8), wd1(8), None, AOP.mult
                    )
                # DVE adds into q0
                for k in (1, 2, 3):
                    nc.vector.tensor_add(
                        qt[0][0:PP, :], qt[0][0:PP, :], qt[k][0:PP, :]
                    )

                if tt_tile:
                    for k in (4, 5, 6, 7, 8):
                        nc.vector.tensor_add(
                            qt[0][0:PP, :], qt[0][0:PP, :], qt[k][0:PP, :]
                        )
                else:
                    def dma_acc(dst, src):
                        nc.gpsimd.dma_start(
                            dst[0:PP, :], src[0:PP, :], accum_op=AOP.add
                        )

                    # DMA-accum tree; 0+=8 runs early (parallel to the q4
                    # side); q0 takes 3 serial accums but q4/q6 free earlier.
                    dma_acc(qt[4], qt[5])
                    dma_acc(qt[6], qt[7])
                    dma_acc(qt[0], qt[8])
                    dma_acc(qt[0], qt[4])
                    dma_acc(qt[0], qt[6])
                # relu into o2 (the only o2 write)
                nc.vector.tensor_scalar_max(acc[0:PP, :], qt[0][0:PP, :], 0.0)

            def dw_g(n, m):
                dw_g_half(n, m, 0)
                dw_g_half(n, m, 1)

            def dw(n):
                """emit dw tiles for image n; pair tile alongside even images
                (its conv1 runs right after the odd image's)."""
                keys = [(n, m) for m in range(4)]
                if n % 2 == 0:
                    keys.append((n, 4))
                # DVE-route first: their products/DMA tails overlap PE matmuls
                for key in keys:
                    if key not in PE_TILES:
                        dw_g(*key)
                for key in keys:
                    if key in PE_TILES:
                        dw_pe(*key)

            def conv3(n):
                """1x1 project + bias, residual via eye96 PSUM preload;
                eviction + bias on Pool."""
                half = 0 if n % 2 == 0 else 64
                for j in range(NCH):
                    ps = ps3p.tile([128, CHUNK], F32, name="ps3", tag="ps3")
                    nc.tensor.matmul(
                        ps[0:C, :],
                        eye96[:, :],
                        x16[n][:, j * CHUNK : (j + 1) * CHUNK],
                        start=True, stop=False,
                    )
                    for kt in range(4):
                        nc.tensor.matmul(
                            ps[0:C, :],
                            w3t[0:128, kt * C : kt * C + C],
                            o2[(kt, n % 2)][0:128, j * CHUNK : (j + 1) * CHUNK],
                            start=False, stop=False,
                        )
                    nc.tensor.matmul(
                        ps[0:C, :],
                        w3t[half : half + 64, 4 * C : 4 * C + C],
                        o2[(4, (n // 2) % 2)][
                            half : half + 64, j * CHUNK : (j + 1) * CHUNK
                        ],
                        start=False, stop=True,
                    )
                    o3 = yp.tile([C, CHUNK], F16, name="o3", tag="o3")
                    # GPSIMD cannot read PSUM on real HW; DVE early (its
                    # mid-pipeline dip), ACT for the tail images (ACT drains
                    # first).
                    if n < 2:
                        nc.vector.tensor_scalar(
                            o3[:], ps[0:C, :], b3c[:], None, AOP.add
                        )
                    else:
                        nc.scalar.activation(
                            o3[:], ps[0:C, :], AF.Identity, bias=b3c[:]
                        )
                    nc.sync.dma_start(
                        y_d[n, :, j * CHUNK : (j + 1) * CHUNK], o3[:]
                    )

            # ---- pipeline ----------------------------------------------------
            # G-routed ctiles first per image: their evicts unblock DVE early
            def c1_order(n):
                ms = list(range(4))
                return sorted(ms, key=lambda m: (n, m) in PE_TILES)

            for ma, mb in zip(c1_order(0), c1_order(1)):
                conv1(0, ma)
                conv1(1, mb)
            conv1(0, 4)
            dw(0)
            for m in c1_order(2):
                conv1(2, m)
            dw(1)
            conv3(0)
            load_x16(3)
            for m in c1_order(3):
                conv1(3, m)
            conv1(2, 4)
            dw(2)           # PE: (2,0),(2,1); G: (2,2),(2,3) + pair(2,4)
            conv3(1)
            dw(3)           # PE: (3,0),(3,1),(3,2); G: (3,3)
            conv3(2)
            conv3(3)

    nc.compile()
    return nc


def _fold_bn(inputs):
    """fold BN params into conv weights/biases; build device-side arrays"""
    f = lambda k: np.asarray(inputs[k], np.float32)
    w1, g1, b1, m1, v1 = f("w1"), f("g1"), f("b1"), f("m1"), f("v1")
    wd, g2, b2, m2, v2 = f("wd"), f("g2"), f("b2"), f("m2"), f("v2")
    w3, g3, b3, m3, v3 = f("w3"), f("g3"), f("b3"), f("m3"), f("v3")

    s1 = g1 / np.sqrt(v1 + EPS)
    W1p = w1[:, :, 0, 0] * s1[:, None]              # [M, C]
    b1p = b1 - m1 * s1                              # [M]
    s2 = g2 / np.sqrt(v2 + EPS)
    wdp = wd[:, 0] * s2[:, None, None]              # [M, 3, 3]
    b2p = b2 - m2 * s2
    s3 = g3 / np.sqrt(v3 + EPS)
    W3p = w3[:, :, 0, 0] * s3[:, None]              # [C, M]
    b3p = b3 - m3 * s3

    w1t = np.ascontiguousarray(W1p.T).astype(np.float16)  # [C, M]

    dgm = np.zeros((128, len(PE_DG_CTILES) * 9 * 128), np.float32)
    wds = np.zeros((128, NCT * 9), np.float32)
    for m, (cs, P) in enumerate(CTILES):
        for k in range(9):
            ky, kx = divmod(k, 3)
            w = wdp[cs : cs + P, ky, kx]
            if m == 4:
                wds[0:64, m * 9 + k] = w
                wds[64:128, m * 9 + k] = w
            else:
                wds[:P, m * 9 + k] = w
            if m in PE_DG_CTILES:
                di = PE_DG_CTILES.index(m)
                if m == 4:
                    blk = dgm[:128, (di * 9 + k) * 128 : (di * 9 + k) * 128 + 128]
                    np.fill_diagonal(blk, np.concatenate([w, w]))
                else:
                    blk = dgm[:P, (di * 9 + k) * 128 : (di * 9 + k) * 128 + P]
                    np.fill_diagonal(blk, w)
    dgm = dgm.astype(np.float16)

    w3t = np.zeros((128, NCT * C), np.float32)
    for kt, (ks, K) in enumerate(CTILES):
        w3t[:K, kt * C : kt * C + C] = W3p.T[ks : ks + K, :]
        if kt == 4:
            w3t[64:128, kt * C : kt * C + C] = W3p.T[ks : ks + K, :]
    w3t = w3t.astype(np.float16)

    b1c = np.zeros((128, NCT), np.float32)
    b2c = np.zeros((128, NCT), np.float32)
    for m, (cs, P) in enumerate(CTILES):
        b1c[:P, m] = b1p[cs : cs + P]
        b2c[:P, m] = b2p[cs : cs + P]
        if m == 4:
            b1c[64:128, m] = b1p[cs : cs + P]
            b2c[64:128, m] = b2p[cs : cs + P]
    b3c = b3p.reshape(C, 1).astype(np.float32)
    eye = np.eye(C, dtype=np.float32).astype(np.float16)

    return dict(
        w1t=w1t, dg=dgm, wds=wds, w3t=w3t, b1c=b1c, b2c=b2c, b3c=b3c, eye96=eye
    )


def _in_maps(inputs):
    params = _fold_bn(inputs)
    x = np.asarray(inputs["x"], np.float32)
    xr = x.reshape(N_CORES, NIMG, C, PIX)
    return [
        dict(
            x16=np.ascontiguousarray(xr[c]).astype(np.float16),
            **params,
        )
        for c in range(N_CORES)
    ]


def kernel(**inputs):
    if "nc" not in _CACHE:
        _CACHE["nc"] = _build()
    nc = _CACHE["nc"]

    B = np.asarray(inputs["x"]).shape[0]
    in_maps = _in_maps(inputs)
    res = run_bass_kernel_spmd(nc, in_maps, core_ids=list(range(N_CORES)))
    out = np.stack(
        [np.asarray(res.results[c]["y"]).astype(np.float32) for c in range(N_CORES)]
    )
    return out.reshape(B, C, H, W)


if __name__ == "__main__":
    rng = np.random.default_rng(0)
    inputs = dict(
        x=rng.standard_normal((32, C, H, W), dtype=np.float32),
        w1=(rng.standard_normal((M, C, 1, 1)) * 0.05).astype(np.float32),
        g1=np.ones(M, np.float32), b1=np.zeros(M, np.float32),
        m1=(rng.standard_normal(M) * 0.1).astype(np.float32),
        v1=rng.uniform(0.5, 1.5, M).astype(np.float32),
        wd=(rng.standard_normal((M, 1, 3, 3)) * 0.1).astype(np.float32),
        g2=np.ones(M, np.float32), b2=np.zeros(M, np.float32),
        m2=(rng.standard_normal(M) * 0.1).astype(np.float32),
        v2=rng.uniform(0.5, 1.5, M).astype(np.float32),
        w3=(rng.standard_normal((C, M, 1, 1)) * 0.05).astype(np.float32),
        g3=np.ones(C, np.float32), b3=np.zeros(C, np.float32),
        m3=np.zeros(C, np.float32), v3=np.ones(C, np.float32),
    )
    out = kernel(**inputs)
    print("kernel out", out.shape, out.dtype)


# revision 62
# speedup vs baseline: 1.3928x; 1.0144x over previous
"""MBConv (expand 1x1 + BN/ReLU, depthwise 3x3 + BN/ReLU, project 1x1 + BN,
residual) on 8 Trainium2 NeuronCores, data-parallel over the batch.

v3 strategy (vs v2 at 261.5us):
- f16 activations/weights everywhere (same cost-model speed as bf16 in every
  path: matmul 1 cyc/row, DVE 4x/2x modes, ACT/Pool/DMA unchanged) for ~8x
  better mantissa -> headroom under the 2e-2 gate.
- conv1 (PE) with ALL evictions on ACT (fused relu+bias, 2-chunk groups).
- dw split by tile route, balanced via LP against the TimelineSim cost model:
    * PE route (7 tiles): 9 diag-matmuls/chunk accumulating in PSUM
      (188ns/chunk-tap at full clock), ACT evict relu+bias.
    * DVE route (11 tiles): tap products via DVE tensor_scalar (877ns/pass,
      4x mode) -- bias rides the first product; 3 products merged with DVE
      tensor_tensor adds (1694ns); 4-5 products merged via gpsimd
      DMA-accumulate tree (Pool desc-gen ~1us + DMA ~1.1us per half);
      1-2 products per tile on ACT (2798ns); final relu on DVE.
- conv3 (PE): residual folded via eye96 @ x16 PSUM preload; eviction + bias
  on Pool (gpsimd tensor_scalar, 717ns/chunk).
- Engine budgets ~PE 140 / DVE 137 / ACT 138 / Pool 138 / DMA 134 us.
"""

import sys

for _p in ("/opt/trn_rl_repo", "/root/.axon_site/_ro/trn_rl_repo"):
    if _p not in sys.path:
        sys.path.append(_p)

import numpy as np

import concourse.bass as bass
import concourse.mybir as mybir
import concourse.tile as tile
from concourse import bacc
from concourse.bass_utils import run_bass_kernel_spmd

EPS = 1e-5
N_CORES = 8
NIMG = 4            # images per core
C = 96              # in/out channels
M = 576             # expanded channels
H = W = 56
PIX = H * W         # 3136
WP = 58             # padded width/height
PPIX = WP * WP      # 3364
RCH = 8             # rows per chunk
CHUNK = RCH * W     # 448
NCH = H // RCH      # 7 chunks
HF = PIX // 2       # half-image cols for accum DMAs (inner run <= 2048)
CTILES = [(0, 128), (128, 128), (256, 128), (384, 128), (512, 64)]
NCT = len(CTILES)

F32 = mybir.dt.float32
F16 = mybir.dt.float16
AOP = mybir.AluOpType
AF = mybir.ActivationFunctionType

# ---- route tables ----------------------------------------------------------
# dw tiles are keyed (img, ctile) for ctile<4 and (pair_img, 4) with
# pair_img in {0, 2} for the image-paired 64-channel ctile.
# PE-routed tiles need diag weights staged; biased toward later images so
# DVE-route work front-loads and drains before the final conv3s.
PE_TILES = {(0, 0), (0, 1), (0, 4), (1, 0), (1, 1), (2, 0), (2, 4), (3, 0)}
PE_DG_CTILES = sorted({m for (_n, m) in PE_TILES})
# tiles whose tap 8 also goes to ACT (2 ACT products instead of 1)
ACT2_TILES = {(2, 2), (2, 3), (3, 1), (3, 2), (3, 3)}
# tiles whose tap 7 goes to DVE instead of ACT (phase-0 ACT relief)
DVE7_TILES = set()
# tiles merged entirely on DVE/ACT (TT adds, no DMA) -- for the tail, where
# DVE is idle and the DMA-accum latency chain would gate the last conv3
TT_TILES = {(3, 3)}
HROWS = 28          # rows per G-tile half (dw processed in row-halves)

_CACHE = {}


def _build(nimg=NIMG):
    nc = bacc.Bacc("TRN2", target_bir_lowering=False, debug=False)

    # ---- dram tensors -----------------------------------------------------
    x16_d = nc.dram_tensor("x16", [nimg, C, PIX], F16, kind="ExternalInput").ap()
    w1t_d = nc.dram_tensor("w1t", [C, M], F16, kind="ExternalInput").ap()
    dg_d = nc.dram_tensor(
        "dg", [128, len(PE_DG_CTILES) * 9 * 128], F16, kind="ExternalInput"
    ).ap()
    wds_d = nc.dram_tensor("wds", [128, NCT * 9], F32, kind="ExternalInput").ap()
    w3t_d = nc.dram_tensor("w3t", [128, NCT * C], F16, kind="ExternalInput").ap()
    b1_d = nc.dram_tensor("b1c", [128, NCT], F32, kind="ExternalInput").ap()
    b2_d = nc.dram_tensor("b2c", [128, NCT], F32, kind="ExternalInput").ap()
    b3_d = nc.dram_tensor("b3c", [C, 1], F32, kind="ExternalInput").ap()
    eye_d = nc.dram_tensor("eye96", [C, C], F16, kind="ExternalInput").ap()
    y_d = nc.dram_tensor("y", [nimg, C, PIX], F16, kind="ExternalOutput").ap()

    with tile.TileContext(nc) as tc:
        with (
            tc.tile_pool(name="const", bufs=1) as constp,
            tc.tile_pool(name="o1", bufs=1) as o1p,
            tc.tile_pool(name="o2", bufs=1) as o2p,
            tc.tile_pool(name="xin", bufs=1) as xp,
            tc.tile_pool(name="scr", bufs=1) as scrp,
            tc.tile_pool(name="ysb", bufs=3) as yp,
            tc.tile_pool(name="ps1", bufs=2, space="PSUM") as ps1p,
            tc.tile_pool(name="psd", bufs=2, space="PSUM") as psdp,
            tc.tile_pool(name="ps3", bufs=2, space="PSUM") as ps3p,
        ):
            # ---- constants / weights -------------------------------------
            w1t = constp.tile([C, M], F16)
            nc.sync.dma_start(w1t[:], w1t_d[:])
            b1c = constp.tile([128, NCT], F32)
            nc.sync.dma_start(b1c[:], b1_d[:])
            # x buffers: 3 rotating, image 3 reuses image 0's slot.
            x16 = []
            for n in range(nimg):
                t = xp.tile([C, PIX], F16, name=f"x16_{n % 3}", tag=f"x{n % 3}")
                x16.append(t)

            def load_x16(n):
                # column-split: first half unblocks conv1 chunks 0-3 early
                for q in range(2):
                    nc.sync.dma_start(
                        x16[n][:, q * HF : (q + 1) * HF],
                        x16_d[n, :, q * HF : (q + 1) * HF],
                    )

            load_x16(0)
            # PE clock warmup while weights load (ramp to full pstate)
            warm = ps1p.tile([C, 1024], F32, name="warm", tag="ps1")
            for _ in range(10):
                nc.tensor.matmul(
                    warm[:, 0:448], w1t[:, 0:C], w1t[:, 0:448],
                    start=True, stop=True,
                )
            dg = constp.tile([128, len(PE_DG_CTILES) * 9 * 128], F16)
            for i in range(len(PE_DG_CTILES)):
                nc.sync.dma_start(
                    dg[:, i * 9 * 128 : (i + 1) * 9 * 128],
                    dg_d[:, i * 9 * 128 : (i + 1) * 9 * 128],
                )
            wds = constp.tile([128, NCT * 9], F32)
            nc.sync.dma_start(wds[:], wds_d[:])
            w3t = constp.tile([128, NCT * C], F16)
            nc.sync.dma_start(w3t[:], w3t_d[:])
            b2c = constp.tile([128, NCT], F32)
            nc.sync.dma_start(b2c[:], b2_d[:])
            b3c = constp.tile([C, 1], F32)
            nc.sync.dma_start(b3c[:], b3_d[:])
            eye96 = constp.tile([C, C], F16)
            nc.sync.dma_start(eye96[:], eye_d[:])
            for n in range(1, 3):
                load_x16(n)

            # ---- persistent activation buffers ---------------------------
            # o1: padded conv1 outputs. ctiles 0-3: slot = n % 2.
            # ctile 4: image-paired, slot = (n // 2) % 2.
            o1 = {}
            o2 = {}
            for m in range(NCT):
                for s in range(2):
                    t1 = o1p.tile([128, PPIX], F16, name=f"o1_{m}_{s}")
                    t1r = t1.rearrange("p (r c) -> p r c", c=WP)
                    nc.gpsimd.memset(t1[:, 0:WP], 0.0)
                    nc.gpsimd.memset(t1[:, PPIX - WP : PPIX], 0.0)
                    nc.gpsimd.memset(t1r[:, :, 0:1], 0.0)
                    nc.gpsimd.memset(t1r[:, :, WP - 1 : WP], 0.0)
                    o1[(m, s)] = t1
                    o2[(m, s)] = o2p.tile([128, PIX], F16, name=f"o2_{m}_{s}")

            def slot(n, m):
                return (n // 2) % 2 if m == 4 else n % 2

            # ------------------------------------------------------------------
            # stage emitters
            # ------------------------------------------------------------------
            def conv1(n, m):
                """conv1 ctile m of image n (ctile 4: n and n+1 paired).
                Evictions on ACT (fused relu + bias); images 0-1 alternate
                ACT/Pool per group (early ACT is eviction-throughput-bound)."""
                cs, P = CTILES[m]
                s = slot(n, m)
                o1r = o1[(m, s)].rearrange("p (r c) -> p r c", c=WP)
                for j0 in range(0, NCH, 2):
                    npair = min(2, NCH - j0)
                    ps = ps1p.tile([128, 1024], F32, name="ps1", tag="ps1")
                    for g in range(npair):
                        c0 = (j0 + g) * CHUNK
                        if m == 4:
                            nc.tensor.matmul(
                                ps[0:64, g * 512 : g * 512 + CHUNK],
                                w1t[:, cs : cs + 64],
                                x16[n][:, c0 : c0 + CHUNK],
                                start=True, stop=True,
                            )
                            nc.tensor.matmul(
                                ps[64:128, g * 512 : g * 512 + CHUNK],
                                w1t[:, cs : cs + 64],
                                x16[n + 1][:, c0 : c0 + CHUNK],
                                start=True, stop=True,
                            )
                        else:
                            nc.tensor.matmul(
                                ps[0:P, g * 512 : g * 512 + CHUNK],
                                w1t[:, cs : cs + P],
                                x16[n][:, c0 : c0 + CHUNK],
                                start=True, stop=True,
                            )
                    PP = 128 if m == 4 else P
                    if npair == 2:
                        src = (
                            ps.rearrange("p (g x) -> p g x", g=2)[0:PP, :, 0:CHUNK]
                            .rearrange("p g (r c) -> p g r c", c=56)
                        )
                        dst = o1r[
                            0:PP, j0 * RCH + 1 : j0 * RCH + 2 * RCH + 1, 1:57
                        ].rearrange("p (g r) c -> p g r c", g=2)
                    else:
                        src = ps[0:PP, 0:CHUNK]
                        dst = o1r[0:PP, j0 * RCH + 1 : j0 * RCH + RCH + 1, 1:57]
                    nc.scalar.activation(
                        dst, src, AF.Relu, bias=b1c[0:PP, m : m + 1]
                    )

            def dw_pe(n, m):
                """depthwise via 9 diag matmuls/chunk on PE (ctile 4: paired,
                full 128 partitions via duplicated diag blocks); ACT evict."""
                cs, P = CTILES[m]
                PP = 128 if m == 4 else P
                s = slot(n, m)
                o1r = o1[(m, s)].rearrange("p (r c) -> p r c", c=WP)
                o2t = o2[(m, s)]
                di = PE_DG_CTILES.index(m)
                for j in range(NCH):
                    ps = psdp.tile([128, CHUNK], F32, name="psd", tag="psd")
                    for k in range(9):
                        ky, kx = divmod(k, 3)
                        nc.tensor.matmul(
                            ps[0:PP, :],
                            dg[0:PP, (di * 9 + k) * 128 : (di * 9 + k) * 128 + PP],
                            o1r[0:PP, j * RCH + ky : j * RCH + ky + 8, kx : kx + 56],
                            start=(k == 0),
                            stop=(k == 8),
                        )
                    nc.scalar.activation(
                        o2t[0:PP, j * CHUNK : (j + 1) * CHUNK],
                        ps[0:PP, :], AF.Relu, bias=b2c[0:PP, m : m + 1],
                    )

            def dw_g_half(n, m, h):
                """one row-half of a DVE-route depthwise tile.

                All products and merges land in scratch; o2 is written only by
                the final relu pass (keeps the write-after-read window vs
                conv3 of image n-2 to a single 468ns op). Merge: 3 DVE
                tensor_tensor adds into q0, 5 gpsimd DMA-accumulates
                (single-DMA per add: inner run 1568 elems < 2048 so no
                descriptor splitting corruption).
                """
                cs, P = CTILES[m]
                s = slot(n, m)
                o1r = o1[(m, s)].rearrange("p (r c) -> p r c", c=WP)
                acc = o2[(m, s)][:, h * HROWS * W : (h + 1) * HROWS * W]
                PP = 128 if m == 4 else P
                two_act = (n, m) in ACT2_TILES

                def tap(k):
                    ky, kx = divmod(k, 3)
                    return o1r[
                        0:PP, h * HROWS + ky : h * HROWS + ky + HROWS, kx : kx + 56
                    ]

                wd1 = lambda k: wds[0:PP, m * 9 + k : m * 9 + k + 1]
                HP = HROWS * W

                qt = {}

                def q(k, tag, bufs):
                    qt[k] = scrp.tile([128, HP], F16, name=tag, tag=tag, bufs=bufs)
                    return qt[k]

                # products (back-to-back to free the o1 slot early)
                nc.vector.tensor_scalar(
                    q(0, "qe", 4)[0:PP, :], tap(0), wd1(0), b2c[0:PP, m : m + 1],
                    AOP.mult, AOP.add,
                )
                for k in (1, 2, 3):   # DVE -> TT-added into q0
                    nc.vector.tensor_scalar(
                        q(k, "qa", 2)[0:PP, :], tap(k), wd1(k), None, AOP.mult
                    )
                tt_tile = (n, m) in TT_TILES
                for k in (4, 5, 6):   # DVE -> DMA-merged (or TT for TT_TILES)
                    nc.vector.tensor_scalar(
                        q(k, "qb", 5)[0:PP, :], tap(k), wd1(k), None, AOP.mult
                    )
                # ACT products (tap 7 unless DVE7; tap 8 if two_act else DVE)
                if (n, m) in DVE7_TILES:
                    nc.vector.tensor_scalar(
                        q(7, "qc", 2)[0:PP, :], tap(7), wd1(7), None, AOP.mult
                    )
                else:
                    nc.scalar.activation(
                        q(7, "qc", 2)[0:PP, :], tap(7), AF.Copy, scale=wd1(7)
                    )
                if two_act:
                    nc.scalar.activation(
                        q(8, "qd", 3)[0:PP, :], tap(8), AF.Copy, scale=wd1(8)
                    )
                else:
                    nc.vector.tensor_scalar(
                        q(8, "qd", 3)[0:PP, :], tap(8), wd1(8), None, AOP.mult
                    )
                # DVE adds into q0
                for k in (1, 2, 3):
                    nc.vector.tensor_add(
                        qt[0][0:PP, :], qt[0][0:PP, :], qt[k][0:PP, :]
                    )

                if tt_tile:
                    for k in (4, 5, 6, 7, 8):
                        nc.vector.tensor_add(
                            qt[0][0:PP, :], qt[0][0:PP, :], qt[k][0:PP, :]
                        )
                else:
                    def dma_acc(dst, src):
                        nc.gpsimd.dma_start(
                            dst[0:PP, :], src[0:PP, :], accum_op=AOP.add
                        )

                    # DMA-accum tree; 0+=8 runs early (parallel to the q4
                    # side); q0 takes 3 serial accums but q4/q6 free earlier.
                    dma_acc(qt[4], qt[5])
                    dma_acc(qt[6], qt[7])
                    dma_acc(qt[0], qt[8])
                    dma_acc(qt[0], qt[4])
                    dma_acc(qt[0], qt[6])
                # relu into o2 (the only o2 write)
                nc.vector.tensor_scalar_max(acc[0:PP, :], qt[0][0:PP, :], 0.0)

            def dw_g(n, m):
                dw_g_half(n, m, 0)
                dw_g_half(n, m, 1)

            def dw(n):
                """emit dw tiles for image n; pair tile alongside even images
                (its conv1 runs right after the odd image's)."""
                keys = [(n, m) for m in range(4)]
                if n % 2 == 0:
                    keys.append((n, 4))
                # DVE-route first: their products/DMA tails overlap PE matmuls
                for key in keys:
                    if key not in PE_TILES:
                        dw_g(*key)
                for key in keys:
                    if key in PE_TILES:
                        dw_pe(*key)

            def conv3(n):
                """1x1 project + bias, residual via eye96 PSUM preload;
                eviction + bias on Pool."""
                half = 0 if n % 2 == 0 else 64
                for j in range(NCH):
                    ps = ps3p.tile([128, CHUNK], F32, name="ps3", tag="ps3")
                    nc.tensor.matmul(
                        ps[0:C, :],
                        eye96[:, :],
                        x16[n][:, j * CHUNK : (j + 1) * CHUNK],
                        start=True, stop=False,
                    )
                    for kt in range(4):
                        nc.tensor.matmul(
                            ps[0:C, :],
                            w3t[0:128, kt * C : kt * C + C],
                            o2[(kt, n % 2)][0:128, j * CHUNK : (j + 1) * CHUNK],
                            start=False, stop=False,
                        )
                    nc.tensor.matmul(
                        ps[0:C, :],
                        w3t[half : half + 64, 4 * C : 4 * C + C],
                        o2[(4, (n // 2) % 2)][
                            half : half + 64, j * CHUNK : (j + 1) * CHUNK
                        ],
                        start=False, stop=True,
                    )
                    o3 = yp.tile([C, CHUNK], F16, name="o3", tag="o3")
                    # GPSIMD cannot read PSUM on real HW; DVE early (its
                    # mid-pipeline dip), ACT for the tail images (ACT drains
                    # first).
                    if n < 2:
                        nc.vector.tensor_scalar(
                            o3[:], ps[0:C, :], b3c[:], None, AOP.add
                        )
                    else:
                        nc.scalar.activation(
                            o3[:], ps[0:C, :], AF.Identity, bias=b3c[:]
                        )
                    nc.sync.dma_start(
                        y_d[n, :, j * CHUNK : (j + 1) * CHUNK], o3[:]
                    )

            # ---- pipeline ----------------------------------------------------
            # G-routed ctiles first per image: their evicts unblock DVE early
            def c1_order(n):
                ms = list(range(4))
                return sorted(ms, key=lambda m: (n, m) in PE_TILES)

            for ma, mb in zip(c1_order(0), c1_order(1)):
                conv1(0, ma)
                conv1(1, mb)
            conv1(0, 4)
            dw(0)
            for m in c1_order(2):
                conv1(2, m)
            dw(1)
            conv3(0)
            load_x16(3)
            for m in c1_order(3):
                conv1(3, m)
            conv1(2, 4)
            dw(2)           # PE: (2,0),(2,1); G: (2,2),(2,3) + pair(2,4)
            conv3(1)
            dw(3)           # PE: (3,0),(3,1),(3,2); G: (3,3)
            conv3(2)
            conv3(3)

    nc.compile()
    return nc


def _fold_bn(inputs):
    """fold BN params into conv weights/biases; build device-side arrays"""
    f = lambda k: np.asarray(inputs[k], np.float32)
    w1, g1, b1, m1, v1 = f("w1"), f("g1"), f("b1"), f("m1"), f("v1")
    wd, g2, b2, m2, v2 = f("wd"), f("g2"), f("b2"), f("m2"), f("v2")
    w3, g3, b3, m3, v3 = f("w3"), f("g3"), f("b3"), f("m3"), f("v3")

    s1 = g1 / np.sqrt(v1 + EPS)
    W1p = w1[:, :, 0, 0] * s1[:, None]              # [M, C]
    b1p = b1 - m1 * s1                              # [M]
    s2 = g2 / np.sqrt(v2 + EPS)
    wdp = wd[:, 0] * s2[:, None, None]              # [M, 3, 3]
    b2p = b2 - m2 * s2
    s3 = g3 / np.sqrt(v3 + EPS)
    W3p = w3[:, :, 0, 0] * s3[:, None]              # [C, M]
    b3p = b3 - m3 * s3

    w1t = np.ascontiguousarray(W1p.T).astype(np.float16)  # [C, M]

    dgm = np.zeros((128, len(PE_DG_CTILES) * 9 * 128), np.float32)
    wds = np.zeros((128, NCT * 9), np.float32)
    for m, (cs, P) in enumerate(CTILES):
        for k in range(9):
            ky, kx = divmod(k, 3)
            w = wdp[cs : cs + P, ky, kx]
            if m == 4:
                wds[0:64, m * 9 + k] = w
                wds[64:128, m * 9 + k] = w
            else:
                wds[:P, m * 9 + k] = w
            if m in PE_DG_CTILES:
                di = PE_DG_CTILES.index(m)
                if m == 4:
                    blk = dgm[:128, (di * 9 + k) * 128 : (di * 9 + k) * 128 + 128]
                    np.fill_diagonal(blk, np.concatenate([w, w]))
                else:
                    blk = dgm[:P, (di * 9 + k) * 128 : (di * 9 + k) * 128 + P]
                    np.fill_diagonal(blk, w)
    dgm = dgm.astype(np.float16)

    w3t = np.zeros((128, NCT * C), np.float32)
    for kt, (ks, K) in enumerate(CTILES):
        w3t[:K, kt * C : kt * C + C] = W3p.T[ks : ks + K, :]
        if kt == 4:
            w3t[64:128, kt * C : kt * C + C] = W3p.T[ks : ks + K, :]
    w3t = w3t.astype(np.float16)

    b1c = np.zeros((128, NCT), np.float32)
    b2c = np.zeros((128, NCT), np.float32)
    for m, (cs, P) in enumerate(CTILES):
        b1c[:P, m] = b1p[cs : cs + P]
        b2c[:P, m] = b2p[cs : cs + P]
        if m == 4:
            b1c[64:128, m] = b1p[cs : cs + P]
            b2c[64:128, m] = b2p[cs : cs + P]
    b3c = b3p.reshape(C, 1).astype(np.float32)
    eye = np.eye(C, dtype=np.float32).astype(np.float16)

    return dict(
        w1t=w1t, dg=dgm, wds=wds, w3t=w3t, b1c=b1c, b2c=b2c, b3c=b3c, eye96=eye
    )


def _in_maps(inputs):
    params = _fold_bn(inputs)
    x = np.asarray(inputs["x"], np.float32)
    xr = x.reshape(N_CORES, NIMG, C, PIX)
    return [
        dict(
            x16=np.ascontiguousarray(xr[c]).astype(np.float16),
            **params,
        )
        for c in range(N_CORES)
    ]


def kernel(**inputs):
    if "nc" not in _CACHE:
        _CACHE["nc"] = _build()
    nc = _CACHE["nc"]

    B = np.asarray(inputs["x"]).shape[0]
    in_maps = _in_maps(inputs)
    res = run_bass_kernel_spmd(nc, in_maps, core_ids=list(range(N_CORES)))
    out = np.stack(
        [np.asarray(res.results[c]["y"]).astype(np.float32) for c in range(N_CORES)]
    )
    return out.reshape(B, C, H, W)


if __name__ == "__main__":
    rng = np.random.default_rng(0)
    inputs = dict(
        x=rng.standard_normal((32, C, H, W), dtype=np.float32),
        w1=(rng.standard_normal((M, C, 1, 1)) * 0.05).astype(np.float32),
        g1=np.ones(M, np.float32), b1=np.zeros(M, np.float32),
        m1=(rng.standard_normal(M) * 0.1).astype(np.float32),
        v1=rng.uniform(0.5, 1.5, M).astype(np.float32),
        wd=(rng.standard_normal((M, 1, 3, 3)) * 0.1).astype(np.float32),
        g2=np.ones(M, np.float32), b2=np.zeros(M, np.float32),
        m2=(rng.standard_normal(M) * 0.1).astype(np.float32),
        v2=rng.uniform(0.5, 1.5, M).astype(np.float32),
        w3=(rng.standard_normal((C, M, 1, 1)) * 0.05).astype(np.float32),
        g3=np.ones(C, np.float32), b3=np.zeros(C, np.float32),
        m3=np.zeros(C, np.float32), v3=np.ones(C, np.float32),
    )
    out = kernel(**inputs)
    print("kernel out", out.shape, out.dtype)


# revision 63
# speedup vs baseline: 1.4094x; 1.0119x over previous
"""MBConv (expand 1x1 + BN/ReLU, depthwise 3x3 + BN/ReLU, project 1x1 + BN,
residual) on 8 Trainium2 NeuronCores, data-parallel over the batch.

v3 strategy (vs v2 at 261.5us):
- f16 activations/weights everywhere (same cost-model speed as bf16 in every
  path: matmul 1 cyc/row, DVE 4x/2x modes, ACT/Pool/DMA unchanged) for ~8x
  better mantissa -> headroom under the 2e-2 gate.
- conv1 (PE) with ALL evictions on ACT (fused relu+bias, 2-chunk groups).
- dw split by tile route, balanced via LP against the TimelineSim cost model:
    * PE route (7 tiles): 9 diag-matmuls/chunk accumulating in PSUM
      (188ns/chunk-tap at full clock), ACT evict relu+bias.
    * DVE route (11 tiles): tap products via DVE tensor_scalar (877ns/pass,
      4x mode) -- bias rides the first product; 3 products merged with DVE
      tensor_tensor adds (1694ns); 4-5 products merged via gpsimd
      DMA-accumulate tree (Pool desc-gen ~1us + DMA ~1.1us per half);
      1-2 products per tile on ACT (2798ns); final relu on DVE.
- conv3 (PE): residual folded via eye96 @ x16 PSUM preload; eviction + bias
  on Pool (gpsimd tensor_scalar, 717ns/chunk).
- Engine budgets ~PE 140 / DVE 137 / ACT 138 / Pool 138 / DMA 134 us.
"""

import sys

for _p in ("/opt/trn_rl_repo", "/root/.axon_site/_ro/trn_rl_repo"):
    if _p not in sys.path:
        sys.path.append(_p)

import numpy as np

import concourse.bass as bass
import concourse.mybir as mybir
import concourse.tile as tile
from concourse import bacc
from concourse.bass_utils import run_bass_kernel_spmd

EPS = 1e-5
N_CORES = 8
NIMG = 4            # images per core
C = 96              # in/out channels
M = 576             # expanded channels
H = W = 56
PIX = H * W         # 3136
WP = 58             # padded width/height
PPIX = WP * WP      # 3364
RCH = 8             # rows per chunk
CHUNK = RCH * W     # 448
NCH = H // RCH      # 7 chunks
HF = PIX // 2       # half-image cols for accum DMAs (inner run <= 2048)
CTILES = [(0, 128), (128, 128), (256, 128), (384, 128), (512, 64)]
NCT = len(CTILES)

F32 = mybir.dt.float32
F16 = mybir.dt.float16
AOP = mybir.AluOpType
AF = mybir.ActivationFunctionType

# ---- route tables ----------------------------------------------------------
# dw tiles are keyed (img, ctile) for ctile<4 and (pair_img, 4) with
# pair_img in {0, 2} for the image-paired 64-channel ctile.
# PE-routed tiles need diag weights staged; biased toward later images so
# DVE-route work front-loads and drains before the final conv3s.
PE_TILES = {(0, 0), (0, 1), (0, 4), (1, 0), (1, 1), (2, 0), (2, 4), (3, 0)}
PE_DG_CTILES = sorted({m for (_n, m) in PE_TILES})
# tiles whose tap 8 also goes to ACT (2 ACT products instead of 1)
ACT2_TILES = {(2, 2), (2, 3), (3, 1), (3, 2), (3, 3)}
# tiles whose tap 7 goes to DVE instead of ACT (phase-0 ACT relief)
DVE7_TILES = {(0, 2), (0, 3)}
# tiles merged entirely on DVE/ACT (TT adds, no DMA) -- for the tail, where
# DVE is idle and the DMA-accum latency chain would gate the last conv3
TT_TILES = {(3, 3)}
HROWS = 28          # rows per G-tile half (dw processed in row-halves)

_CACHE = {}


def _build(nimg=NIMG):
    nc = bacc.Bacc("TRN2", target_bir_lowering=False, debug=False)

    # ---- dram tensors -----------------------------------------------------
    x16_d = nc.dram_tensor("x16", [nimg, C, PIX], F16, kind="ExternalInput").ap()
    w1t_d = nc.dram_tensor("w1t", [C, M], F16, kind="ExternalInput").ap()
    dg_d = nc.dram_tensor(
        "dg", [128, len(PE_DG_CTILES) * 9 * 128], F16, kind="ExternalInput"
    ).ap()
    wds_d = nc.dram_tensor("wds", [128, NCT * 9], F32, kind="ExternalInput").ap()
    w3t_d = nc.dram_tensor("w3t", [128, NCT * C], F16, kind="ExternalInput").ap()
    b1_d = nc.dram_tensor("b1c", [128, NCT], F32, kind="ExternalInput").ap()
    b2_d = nc.dram_tensor("b2c", [128, NCT], F32, kind="ExternalInput").ap()
    b3_d = nc.dram_tensor("b3c", [C, 1], F32, kind="ExternalInput").ap()
    eye_d = nc.dram_tensor("eye96", [C, C], F16, kind="ExternalInput").ap()
    y_d = nc.dram_tensor("y", [nimg, C, PIX], F16, kind="ExternalOutput").ap()

    with tile.TileContext(nc) as tc:
        with (
            tc.tile_pool(name="const", bufs=1) as constp,
            tc.tile_pool(name="o1", bufs=1) as o1p,
            tc.tile_pool(name="o2", bufs=1) as o2p,
            tc.tile_pool(name="xin", bufs=1) as xp,
            tc.tile_pool(name="scr", bufs=1) as scrp,
            tc.tile_pool(name="ysb", bufs=3) as yp,
            tc.tile_pool(name="ps1", bufs=2, space="PSUM") as ps1p,
            tc.tile_pool(name="psd", bufs=2, space="PSUM") as psdp,
            tc.tile_pool(name="ps3", bufs=2, space="PSUM") as ps3p,
        ):
            # ---- constants / weights -------------------------------------
            w1t = constp.tile([C, M], F16)
            nc.sync.dma_start(w1t[:], w1t_d[:])
            b1c = constp.tile([128, NCT], F32)
            nc.sync.dma_start(b1c[:], b1_d[:])
            # x buffers: 3 rotating, image 3 reuses image 0's slot.
            x16 = []
            for n in range(nimg):
                t = xp.tile([C, PIX], F16, name=f"x16_{n % 3}", tag=f"x{n % 3}")
                x16.append(t)

            def load_x16(n):
                # column-split: first half unblocks conv1 chunks 0-3 early
                for q in range(2):
                    nc.sync.dma_start(
                        x16[n][:, q * HF : (q + 1) * HF],
                        x16_d[n, :, q * HF : (q + 1) * HF],
                    )

            load_x16(0)
            # PE clock warmup while weights load (ramp to full pstate)
            warm = ps1p.tile([C, 1024], F32, name="warm", tag="ps1")
            for _ in range(10):
                nc.tensor.matmul(
                    warm[:, 0:448], w1t[:, 0:C], w1t[:, 0:448],
                    start=True, stop=True,
                )
            dg = constp.tile([128, len(PE_DG_CTILES) * 9 * 128], F16)
            for i in range(len(PE_DG_CTILES)):
                nc.sync.dma_start(
                    dg[:, i * 9 * 128 : (i + 1) * 9 * 128],
                    dg_d[:, i * 9 * 128 : (i + 1) * 9 * 128],
                )
            wds = constp.tile([128, NCT * 9], F32)
            nc.sync.dma_start(wds[:], wds_d[:])
            w3t = constp.tile([128, NCT * C], F16)
            nc.sync.dma_start(w3t[:], w3t_d[:])
            b2c = constp.tile([128, NCT], F32)
            nc.sync.dma_start(b2c[:], b2_d[:])
            b3c = constp.tile([C, 1], F32)
            nc.sync.dma_start(b3c[:], b3_d[:])
            eye96 = constp.tile([C, C], F16)
            nc.sync.dma_start(eye96[:], eye_d[:])
            for n in range(1, 3):
                load_x16(n)

            # ---- persistent activation buffers ---------------------------
            # o1: padded conv1 outputs. ctiles 0-3: slot = n % 2.
            # ctile 4: image-paired, slot = (n // 2) % 2.
            o1 = {}
            o2 = {}
            for m in range(NCT):
                for s in range(2):
                    t1 = o1p.tile([128, PPIX], F16, name=f"o1_{m}_{s}")
                    t1r = t1.rearrange("p (r c) -> p r c", c=WP)
                    nc.gpsimd.memset(t1[:, 0:WP], 0.0)
                    nc.gpsimd.memset(t1[:, PPIX - WP : PPIX], 0.0)
                    nc.gpsimd.memset(t1r[:, :, 0:1], 0.0)
                    nc.gpsimd.memset(t1r[:, :, WP - 1 : WP], 0.0)
                    o1[(m, s)] = t1
                    o2[(m, s)] = o2p.tile([128, PIX], F16, name=f"o2_{m}_{s}")

            def slot(n, m):
                return (n // 2) % 2 if m == 4 else n % 2

            # ------------------------------------------------------------------
            # stage emitters
            # ------------------------------------------------------------------
            def conv1(n, m):
                """conv1 ctile m of image n (ctile 4: n and n+1 paired).
                Evictions on ACT (fused relu + bias); images 0-1 alternate
                ACT/Pool per group (early ACT is eviction-throughput-bound)."""
                cs, P = CTILES[m]
                s = slot(n, m)
                o1r = o1[(m, s)].rearrange("p (r c) -> p r c", c=WP)
                for j0 in range(0, NCH, 2):
                    npair = min(2, NCH - j0)
                    ps = ps1p.tile([128, 1024], F32, name="ps1", tag="ps1")
                    for g in range(npair):
                        c0 = (j0 + g) * CHUNK
                        if m == 4:
                            nc.tensor.matmul(
                                ps[0:64, g * 512 : g * 512 + CHUNK],
                                w1t[:, cs : cs + 64],
                                x16[n][:, c0 : c0 + CHUNK],
                                start=True, stop=True,
                            )
                            nc.tensor.matmul(
                                ps[64:128, g * 512 : g * 512 + CHUNK],
                                w1t[:, cs : cs + 64],
                                x16[n + 1][:, c0 : c0 + CHUNK],
                                start=True, stop=True,
                            )
                        else:
                            nc.tensor.matmul(
                                ps[0:P, g * 512 : g * 512 + CHUNK],
                                w1t[:, cs : cs + P],
                                x16[n][:, c0 : c0 + CHUNK],
                                start=True, stop=True,
                            )
                    PP = 128 if m == 4 else P
                    if npair == 2:
                        src = (
                            ps.rearrange("p (g x) -> p g x", g=2)[0:PP, :, 0:CHUNK]
                            .rearrange("p g (r c) -> p g r c", c=56)
                        )
                        dst = o1r[
                            0:PP, j0 * RCH + 1 : j0 * RCH + 2 * RCH + 1, 1:57
                        ].rearrange("p (g r) c -> p g r c", g=2)
                    else:
                        src = ps[0:PP, 0:CHUNK]
                        dst = o1r[0:PP, j0 * RCH + 1 : j0 * RCH + RCH + 1, 1:57]
                    nc.scalar.activation(
                        dst, src, AF.Relu, bias=b1c[0:PP, m : m + 1]
                    )

            def dw_pe(n, m):
                """depthwise via 9 diag matmuls/chunk on PE (ctile 4: paired,
                full 128 partitions via duplicated diag blocks); ACT evict."""
                cs, P = CTILES[m]
                PP = 128 if m == 4 else P
                s = slot(n, m)
                o1r = o1[(m, s)].rearrange("p (r c) -> p r c", c=WP)
                o2t = o2[(m, s)]
                di = PE_DG_CTILES.index(m)
                for j in range(NCH):
                    ps = psdp.tile([128, CHUNK], F32, name="psd", tag="psd")
                    for k in range(9):
                        ky, kx = divmod(k, 3)
                        nc.tensor.matmul(
                            ps[0:PP, :],
                            dg[0:PP, (di * 9 + k) * 128 : (di * 9 + k) * 128 + PP],
                            o1r[0:PP, j * RCH + ky : j * RCH + ky + 8, kx : kx + 56],
                            start=(k == 0),
                            stop=(k == 8),
                        )
                    nc.scalar.activation(
                        o2t[0:PP, j * CHUNK : (j + 1) * CHUNK],
                        ps[0:PP, :], AF.Relu, bias=b2c[0:PP, m : m + 1],
                    )

            def dw_g_half(n, m, h):
                """one row-half of a DVE-route depthwise tile.

                All products and merges land in scratch; o2 is written only by
                the final relu pass (keeps the write-after-read window vs
                conv3 of image n-2 to a single 468ns op). Merge: 3 DVE
                tensor_tensor adds into q0, 5 gpsimd DMA-accumulates
                (single-DMA per add: inner run 1568 elems < 2048 so no
                descriptor splitting corruption).
                """
                cs, P = CTILES[m]
                s = slot(n, m)
                o1r = o1[(m, s)].rearrange("p (r c) -> p r c", c=WP)
                acc = o2[(m, s)][:, h * HROWS * W : (h + 1) * HROWS * W]
                PP = 128 if m == 4 else P
                two_act = (n, m) in ACT2_TILES

                def tap(k):
                    ky, kx = divmod(k, 3)
                    return o1r[
                        0:PP, h * HROWS + ky : h * HROWS + ky + HROWS, kx : kx + 56
                    ]

                wd1 = lambda k: wds[0:PP, m * 9 + k : m * 9 + k + 1]
                HP = HROWS * W

                qt = {}

                def q(k, tag, bufs):
                    qt[k] = scrp.tile([128, HP], F16, name=tag, tag=tag, bufs=bufs)
                    return qt[k]

                # products (back-to-back to free the o1 slot early)
                nc.vector.tensor_scalar(
                    q(0, "qe", 4)[0:PP, :], tap(0), wd1(0), b2c[0:PP, m : m + 1],
                    AOP.mult, AOP.add,
                )
                for k in (1, 2, 3):   # DVE -> TT-added into q0
                    nc.vector.tensor_scalar(
                        q(k, "qa", 2)[0:PP, :], tap(k), wd1(k), None, AOP.mult
                    )
                tt_tile = (n, m) in TT_TILES
                for k in (4, 5, 6):   # DVE -> DMA-merged (or TT for TT_TILES)
                    nc.vector.tensor_scalar(
                        q(k, "qb", 5)[0:PP, :], tap(k), wd1(k), None, AOP.mult
                    )
                # ACT products (tap 7 unless DVE7; tap 8 if two_act else DVE)
                if (n, m) in DVE7_TILES:
                    nc.vector.tensor_scalar(
                        q(7, "qc", 2)[0:PP, :], tap(7), wd1(7), None, AOP.mult
                    )
                else:
                    nc.scalar.activation(
                        q(7, "qc", 2)[0:PP, :], tap(7), AF.Copy, scale=wd1(7)
                    )
                if two_act:
                    nc.scalar.activation(
                        q(8, "qd", 3)[0:PP, :], tap(8), AF.Copy, scale=wd1(8)
                    )
                else:
                    nc.vector.tensor_scalar(
                        q(8, "qd", 3)[0:PP, :], tap(8), wd1(8), None, AOP.mult
                    )
                # DVE adds into q0
                for k in (1, 2, 3):
                    nc.vector.tensor_add(
                        qt[0][0:PP, :], qt[0][0:PP, :], qt[k][0:PP, :]
                    )

                if tt_tile:
                    for k in (4, 5, 6, 7, 8):
                        nc.vector.tensor_add(
                            qt[0][0:PP, :], qt[0][0:PP, :], qt[k][0:PP, :]
                        )
                else:
                    def dma_acc(dst, src):
                        nc.gpsimd.dma_start(
                            dst[0:PP, :], src[0:PP, :], accum_op=AOP.add
                        )

                    # DMA-accum tree; 0+=8 runs early (parallel to the q4
                    # side); q0 takes 3 serial accums but q4/q6 free earlier.
                    dma_acc(qt[4], qt[5])
                    dma_acc(qt[6], qt[7])
                    dma_acc(qt[0], qt[8])
                    dma_acc(qt[0], qt[4])
                    dma_acc(qt[0], qt[6])
                # relu into o2 (the only o2 write)
                nc.vector.tensor_scalar_max(acc[0:PP, :], qt[0][0:PP, :], 0.0)

            def dw_g(n, m):
                dw_g_half(n, m, 0)
                dw_g_half(n, m, 1)

            def dw(n):
                """emit dw tiles for image n; pair tile alongside even images
                (its conv1 runs right after the odd image's)."""
                keys = [(n, m) for m in range(4)]
                if n % 2 == 0:
                    keys.append((n, 4))
                # DVE-route first: their products/DMA tails overlap PE matmuls
                for key in keys:
                    if key not in PE_TILES:
                        dw_g(*key)
                for key in keys:
                    if key in PE_TILES:
                        dw_pe(*key)

            def conv3(n):
                """1x1 project + bias, residual via eye96 PSUM preload;
                eviction + bias on Pool."""
                half = 0 if n % 2 == 0 else 64
                for j in range(NCH):
                    ps = ps3p.tile([128, CHUNK], F32, name="ps3", tag="ps3")
                    nc.tensor.matmul(
                        ps[0:C, :],
                        eye96[:, :],
                        x16[n][:, j * CHUNK : (j + 1) * CHUNK],
                        start=True, stop=False,
                    )
                    for kt in range(4):
                        nc.tensor.matmul(
                            ps[0:C, :],
                            w3t[0:128, kt * C : kt * C + C],
                            o2[(kt, n % 2)][0:128, j * CHUNK : (j + 1) * CHUNK],
                            start=False, stop=False,
                        )
                    nc.tensor.matmul(
                        ps[0:C, :],
                        w3t[half : half + 64, 4 * C : 4 * C + C],
                        o2[(4, (n // 2) % 2)][
                            half : half + 64, j * CHUNK : (j + 1) * CHUNK
                        ],
                        start=False, stop=True,
                    )
                    o3 = yp.tile([C, CHUNK], F16, name="o3", tag="o3")
                    # GPSIMD cannot read PSUM on real HW; DVE early (its
                    # mid-pipeline dip), ACT for the tail images (ACT drains
                    # first).
                    if n < 2:
                        nc.vector.tensor_scalar(
                            o3[:], ps[0:C, :], b3c[:], None, AOP.add
                        )
                    else:
                        nc.scalar.activation(
                            o3[:], ps[0:C, :], AF.Identity, bias=b3c[:]
                        )
                    nc.sync.dma_start(
                        y_d[n, :, j * CHUNK : (j + 1) * CHUNK], o3[:]
                    )

            # ---- pipeline ----------------------------------------------------
            # G-routed ctiles first per image: their evicts unblock DVE early
            def c1_order(n):
                ms = list(range(4))
                return sorted(ms, key=lambda m: (n, m) in PE_TILES)

            for ma, mb in zip(c1_order(0), c1_order(1)):
                conv1(0, ma)
                conv1(1, mb)
            conv1(0, 4)
            dw(0)
            for m in c1_order(2):
                conv1(2, m)
            dw(1)
            conv3(0)
            load_x16(3)
            for m in c1_order(3):
                conv1(3, m)
            conv1(2, 4)
            dw(2)           # PE: (2,0),(2,1); G: (2,2),(2,3) + pair(2,4)
            conv3(1)
            dw(3)           # PE: (3,0),(3,1),(3,2); G: (3,3)
            conv3(2)
            conv3(3)

    nc.compile()
    return nc


def _fold_bn(inputs):
    """fold BN params into conv weights/biases; build device-side arrays"""
    f = lambda k: np.asarray(inputs[k], np.float32)
    w1, g1, b1, m1, v1 = f("w1"), f("g1"), f("b1"), f("m1"), f("v1")
    wd, g2, b2, m2, v2 = f("wd"), f("g2"), f("b2"), f("m2"), f("v2")
    w3, g3, b3, m3, v3 = f("w3"), f("g3"), f("b3"), f("m3"), f("v3")

    s1 = g1 / np.sqrt(v1 + EPS)
    W1p = w1[:, :, 0, 0] * s1[:, None]              # [M, C]
    b1p = b1 - m1 * s1                              # [M]
    s2 = g2 / np.sqrt(v2 + EPS)
    wdp = wd[:, 0] * s2[:, None, None]              # [M, 3, 3]
    b2p = b2 - m2 * s2
    s3 = g3 / np.sqrt(v3 + EPS)
    W3p = w3[:, :, 0, 0] * s3[:, None]              # [C, M]
    b3p = b3 - m3 * s3

    w1t = np.ascontiguousarray(W1p.T).astype(np.float16)  # [C, M]

    dgm = np.zeros((128, len(PE_DG_CTILES) * 9 * 128), np.float32)
    wds = np.zeros((128, NCT * 9), np.float32)
    for m, (cs, P) in enumerate(CTILES):
        for k in range(9):
            ky, kx = divmod(k, 3)
            w = wdp[cs : cs + P, ky, kx]
            if m == 4:
                wds[0:64, m * 9 + k] = w
                wds[64:128, m * 9 + k] = w
            else:
                wds[:P, m * 9 + k] = w
            if m in PE_DG_CTILES:
                di = PE_DG_CTILES.index(m)
                if m == 4:
                    blk = dgm[:128, (di * 9 + k) * 128 : (di * 9 + k) * 128 + 128]
                    np.fill_diagonal(blk, np.concatenate([w, w]))
                else:
                    blk = dgm[:P, (di * 9 + k) * 128 : (di * 9 + k) * 128 + P]
                    np.fill_diagonal(blk, w)
    dgm = dgm.astype(np.float16)

    w3t = np.zeros((128, NCT * C), np.float32)
    for kt, (ks, K) in enumerate(CTILES):
        w3t[:K, kt * C : kt * C + C] = W3p.T[ks : ks + K, :]
        if kt == 4:
            w3t[64:128, kt * C : kt * C + C] = W3p.T[ks : ks + K, :]
    w3t = w3t.astype(np.float16)

    b1c = np.zeros((128, NCT), np.float32)
    b2c = np.zeros((128, NCT), np.float32)
    for m, (cs, P) in enumerate(CTILES):
        b1c[:P, m] = b1p[cs : cs + P]
        b2c[:P, m] = b2p[cs : cs + P]
        if m == 4:
            b1c[64:128, m] = b1p[cs : cs + P]
            b2c[64:128, m] = b2p[cs : cs + P]
    b3c = b3p.reshape(C, 1).astype(np.float32)
    eye = np.eye(C, dtype=np.float32).astype(np.float16)

    return dict(
        w1t=w1t, dg=dgm, wds=wds, w3t=w3t, b1c=b1c, b2c=b2c, b3c=b3c, eye96=eye
    )


def _in_maps(inputs):
    params = _fold_bn(inputs)
    x = np.asarray(inputs["x"], np.float32)
    xr = x.reshape(N_CORES, NIMG, C, PIX)
    return [
        dict(
            x16=np.ascontiguousarray(xr[c]).astype(np.float16),
            **params,
        )
        for c in range(N_CORES)
    ]


def kernel(**inputs):
    if "nc" not in _CACHE:
        _CACHE["nc"] = _build()
    nc = _CACHE["nc"]

    B = np.asarray(inputs["x"]).shape[0]
    in_maps = _in_maps(inputs)
    res = run_bass_kernel_spmd(nc, in_maps, core_ids=list(range(N_CORES)))
    out = np.stack(
        [np.asarray(res.results[c]["y"]).astype(np.float32) for c in range(N_CORES)]
    )
    return out.reshape(B, C, H, W)


if __name__ == "__main__":
    rng = np.random.default_rng(0)
    inputs = dict(
        x=rng.standard_normal((32, C, H, W), dtype=np.float32),
        w1=(rng.standard_normal((M, C, 1, 1)) * 0.05).astype(np.float32),
        g1=np.ones(M, np.float32), b1=np.zeros(M, np.float32),
        m1=(rng.standard_normal(M) * 0.1).astype(np.float32),
        v1=rng.uniform(0.5, 1.5, M).astype(np.float32),
        wd=(rng.standard_normal((M, 1, 3, 3)) * 0.1).astype(np.float32),
        g2=np.ones(M, np.float32), b2=np.zeros(M, np.float32),
        m2=(rng.standard_normal(M) * 0.1).astype(np.float32),
        v2=rng.uniform(0.5, 1.5, M).astype(np.float32),
        w3=(rng.standard_normal((C, M, 1, 1)) * 0.05).astype(np.float32),
        g3=np.ones(C, np.float32), b3=np.zeros(C, np.float32),
        m3=np.zeros(C, np.float32), v3=np.ones(C, np.float32),
    )
    out = kernel(**inputs)
    print("kernel out", out.shape, out.dtype)


# revision 65
# speedup vs baseline: 1.4182x; 1.0063x over previous
"""MBConv (expand 1x1 + BN/ReLU, depthwise 3x3 + BN/ReLU, project 1x1 + BN,
residual) on 8 Trainium2 NeuronCores, data-parallel over the batch.

v3 strategy (vs v2 at 261.5us):
- f16 activations/weights everywhere (same cost-model speed as bf16 in every
  path: matmul 1 cyc/row, DVE 4x/2x modes, ACT/Pool/DMA unchanged) for ~8x
  better mantissa -> headroom under the 2e-2 gate.
- conv1 (PE) with ALL evictions on ACT (fused relu+bias, 2-chunk groups).
- dw split by tile route, balanced via LP against the TimelineSim cost model:
    * PE route (7 tiles): 9 diag-matmuls/chunk accumulating in PSUM
      (188ns/chunk-tap at full clock), ACT evict relu+bias.
    * DVE route (11 tiles): tap products via DVE tensor_scalar (877ns/pass,
      4x mode) -- bias rides the first product; 3 products merged with DVE
      tensor_tensor adds (1694ns); 4-5 products merged via gpsimd
      DMA-accumulate tree (Pool desc-gen ~1us + DMA ~1.1us per half);
      1-2 products per tile on ACT (2798ns); final relu on DVE.
- conv3 (PE): residual folded via eye96 @ x16 PSUM preload; eviction + bias
  on Pool (gpsimd tensor_scalar, 717ns/chunk).
- Engine budgets ~PE 140 / DVE 137 / ACT 138 / Pool 138 / DMA 134 us.
"""

import sys

for _p in ("/opt/trn_rl_repo", "/root/.axon_site/_ro/trn_rl_repo"):
    if _p not in sys.path:
        sys.path.append(_p)

import numpy as np

import concourse.bass as bass
import concourse.mybir as mybir
import concourse.tile as tile
from concourse import bacc
from concourse.bass_utils import run_bass_kernel_spmd

EPS = 1e-5
N_CORES = 8
NIMG = 4            # images per core
C = 96              # in/out channels
M = 576             # expanded channels
H = W = 56
PIX = H * W         # 3136
WP = 58             # padded width/height
PPIX = WP * WP      # 3364
RCH = 8             # rows per chunk
CHUNK = RCH * W     # 448
NCH = H // RCH      # 7 chunks
HF = PIX // 2       # half-image cols for accum DMAs (inner run <= 2048)
CTILES = [(0, 128), (128, 128), (256, 128), (384, 128), (512, 64)]
NCT = len(CTILES)

F32 = mybir.dt.float32
F16 = mybir.dt.float16
AOP = mybir.AluOpType
AF = mybir.ActivationFunctionType

# ---- route tables ----------------------------------------------------------
# dw tiles are keyed (img, ctile) for ctile<4 and (pair_img, 4) with
# pair_img in {0, 2} for the image-paired 64-channel ctile.
# PE-routed tiles need diag weights staged; biased toward later images so
# DVE-route work front-loads and drains before the final conv3s.
PE_TILES = {(0, 0), (0, 1), (0, 4), (1, 0), (1, 1), (2, 0), (2, 4), (3, 0)}
PE_DG_CTILES = sorted({m for (_n, m) in PE_TILES})
# tiles whose tap 8 also goes to ACT (2 ACT products instead of 1)
ACT2_TILES = {(2, 2), (2, 3), (3, 1), (3, 2), (3, 3)}
# tiles whose tap 7 goes to DVE instead of ACT (phase-0 ACT relief)
DVE7_TILES = {(0, 2), (0, 3), (1, 2), (1, 3), (2, 1)}
# tiles merged entirely on DVE/ACT (TT adds, no DMA) -- for the tail, where
# DVE is idle and the DMA-accum latency chain would gate the last conv3
TT_TILES = {(3, 3)}
HROWS = 28          # rows per G-tile half (dw processed in row-halves)

_CACHE = {}


def _build(nimg=NIMG):
    nc = bacc.Bacc("TRN2", target_bir_lowering=False, debug=False)

    # ---- dram tensors -----------------------------------------------------
    x16_d = nc.dram_tensor("x16", [nimg, C, PIX], F16, kind="ExternalInput").ap()
    w1t_d = nc.dram_tensor("w1t", [C, M], F16, kind="ExternalInput").ap()
    dg_d = nc.dram_tensor(
        "dg", [128, len(PE_DG_CTILES) * 9 * 128], F16, kind="ExternalInput"
    ).ap()
    wds_d = nc.dram_tensor("wds", [128, NCT * 9], F32, kind="ExternalInput").ap()
    w3t_d = nc.dram_tensor("w3t", [128, NCT * C], F16, kind="ExternalInput").ap()
    b1_d = nc.dram_tensor("b1c", [128, NCT], F32, kind="ExternalInput").ap()
    b2_d = nc.dram_tensor("b2c", [128, NCT], F32, kind="ExternalInput").ap()
    b3_d = nc.dram_tensor("b3c", [C, 1], F32, kind="ExternalInput").ap()
    eye_d = nc.dram_tensor("eye96", [C, C], F16, kind="ExternalInput").ap()
    y_d = nc.dram_tensor("y", [nimg, C, PIX], F16, kind="ExternalOutput").ap()

    with tile.TileContext(nc) as tc:
        with (
            tc.tile_pool(name="const", bufs=1) as constp,
            tc.tile_pool(name="o1", bufs=1) as o1p,
            tc.tile_pool(name="o2", bufs=1) as o2p,
            tc.tile_pool(name="xin", bufs=1) as xp,
            tc.tile_pool(name="scr", bufs=1) as scrp,
            tc.tile_pool(name="ysb", bufs=3) as yp,
            tc.tile_pool(name="ps1", bufs=2, space="PSUM") as ps1p,
            tc.tile_pool(name="psd", bufs=2, space="PSUM") as psdp,
            tc.tile_pool(name="ps3", bufs=2, space="PSUM") as ps3p,
        ):
            # ---- constants / weights -------------------------------------
            w1t = constp.tile([C, M], F16)
            nc.sync.dma_start(w1t[:], w1t_d[:])
            b1c = constp.tile([128, NCT], F32)
            nc.sync.dma_start(b1c[:], b1_d[:])
            # x buffers: 3 rotating, image 3 reuses image 0's slot.
            x16 = []
            for n in range(nimg):
                t = xp.tile([C, PIX], F16, name=f"x16_{n % 3}", tag=f"x{n % 3}")
                x16.append(t)

            def load_x16(n):
                # column-split: first half unblocks conv1 chunks 0-3 early
                for q in range(2):
                    nc.sync.dma_start(
                        x16[n][:, q * HF : (q + 1) * HF],
                        x16_d[n, :, q * HF : (q + 1) * HF],
                    )

            load_x16(0)
            # PE clock warmup while weights load (ramp to full pstate)
            warm = ps1p.tile([C, 1024], F32, name="warm", tag="ps1")
            for _ in range(10):
                nc.tensor.matmul(
                    warm[:, 0:448], w1t[:, 0:C], w1t[:, 0:448],
                    start=True, stop=True,
                )
            dg = constp.tile([128, len(PE_DG_CTILES) * 9 * 128], F16)
            for i in range(len(PE_DG_CTILES)):
                nc.sync.dma_start(
                    dg[:, i * 9 * 128 : (i + 1) * 9 * 128],
                    dg_d[:, i * 9 * 128 : (i + 1) * 9 * 128],
                )
            wds = constp.tile([128, NCT * 9], F32)
            nc.sync.dma_start(wds[:], wds_d[:])
            w3t = constp.tile([128, NCT * C], F16)
            nc.sync.dma_start(w3t[:], w3t_d[:])
            b2c = constp.tile([128, NCT], F32)
            nc.sync.dma_start(b2c[:], b2_d[:])
            b3c = constp.tile([C, 1], F32)
            nc.sync.dma_start(b3c[:], b3_d[:])
            eye96 = constp.tile([C, C], F16)
            nc.sync.dma_start(eye96[:], eye_d[:])
            for n in range(1, 3):
                load_x16(n)

            # ---- persistent activation buffers ---------------------------
            # o1: padded conv1 outputs. ctiles 0-3: slot = n % 2.
            # ctile 4: image-paired, slot = (n // 2) % 2.
            o1 = {}
            o2 = {}
            for m in range(NCT):
                for s in range(2):
                    t1 = o1p.tile([128, PPIX], F16, name=f"o1_{m}_{s}")
                    t1r = t1.rearrange("p (r c) -> p r c", c=WP)
                    nc.gpsimd.memset(t1[:, 0:WP], 0.0)
                    nc.gpsimd.memset(t1[:, PPIX - WP : PPIX], 0.0)
                    nc.gpsimd.memset(t1r[:, :, 0:1], 0.0)
                    nc.gpsimd.memset(t1r[:, :, WP - 1 : WP], 0.0)
                    o1[(m, s)] = t1
                    o2[(m, s)] = o2p.tile([128, PIX], F16, name=f"o2_{m}_{s}")

            def slot(n, m):
                return (n // 2) % 2 if m == 4 else n % 2

            # ------------------------------------------------------------------
            # stage emitters
            # ------------------------------------------------------------------
            def conv1(n, m):
                """conv1 ctile m of image n (ctile 4: n and n+1 paired).
                Evictions on ACT (fused relu + bias); images 0-1 alternate
                ACT/Pool per group (early ACT is eviction-throughput-bound)."""
                cs, P = CTILES[m]
                s = slot(n, m)
                o1r = o1[(m, s)].rearrange("p (r c) -> p r c", c=WP)
                for j0 in range(0, NCH, 2):
                    npair = min(2, NCH - j0)
                    ps = ps1p.tile([128, 1024], F32, name="ps1", tag="ps1")
                    for g in range(npair):
                        c0 = (j0 + g) * CHUNK
                        if m == 4:
                            nc.tensor.matmul(
                                ps[0:64, g * 512 : g * 512 + CHUNK],
                                w1t[:, cs : cs + 64],
                                x16[n][:, c0 : c0 + CHUNK],
                                start=True, stop=True,
                            )
                            nc.tensor.matmul(
                                ps[64:128, g * 512 : g * 512 + CHUNK],
                                w1t[:, cs : cs + 64],
                                x16[n + 1][:, c0 : c0 + CHUNK],
                                start=True, stop=True,
                            )
                        else:
                            nc.tensor.matmul(
                                ps[0:P, g * 512 : g * 512 + CHUNK],
                                w1t[:, cs : cs + P],
                                x16[n][:, c0 : c0 + CHUNK],
                                start=True, stop=True,
                            )
                    PP = 128 if m == 4 else P
                    if npair == 2:
                        src = (
                            ps.rearrange("p (g x) -> p g x", g=2)[0:PP, :, 0:CHUNK]
                            .rearrange("p g (r c) -> p g r c", c=56)
                        )
                        dst = o1r[
                            0:PP, j0 * RCH + 1 : j0 * RCH + 2 * RCH + 1, 1:57
                        ].rearrange("p (g r) c -> p g r c", g=2)
                    else:
                        src = ps[0:PP, 0:CHUNK]
                        dst = o1r[0:PP, j0 * RCH + 1 : j0 * RCH + RCH + 1, 1:57]
                    nc.scalar.activation(
                        dst, src, AF.Relu, bias=b1c[0:PP, m : m + 1]
                    )

            def dw_pe(n, m):
                """depthwise via 9 diag matmuls/chunk on PE (ctile 4: paired,
                full 128 partitions via duplicated diag blocks); ACT evict."""
                cs, P = CTILES[m]
                PP = 128 if m == 4 else P
                s = slot(n, m)
                o1r = o1[(m, s)].rearrange("p (r c) -> p r c", c=WP)
                o2t = o2[(m, s)]
                di = PE_DG_CTILES.index(m)
                for j in range(NCH):
                    ps = psdp.tile([128, CHUNK], F32, name="psd", tag="psd")
                    for k in range(9):
                        ky, kx = divmod(k, 3)
                        nc.tensor.matmul(
                            ps[0:PP, :],
                            dg[0:PP, (di * 9 + k) * 128 : (di * 9 + k) * 128 + PP],
                            o1r[0:PP, j * RCH + ky : j * RCH + ky + 8, kx : kx + 56],
                            start=(k == 0),
                            stop=(k == 8),
                        )
                    nc.scalar.activation(
                        o2t[0:PP, j * CHUNK : (j + 1) * CHUNK],
                        ps[0:PP, :], AF.Relu, bias=b2c[0:PP, m : m + 1],
                    )

            def dw_g_half(n, m, h):
                """one row-half of a DVE-route depthwise tile.

                All products and merges land in scratch; o2 is written only by
                the final relu pass (keeps the write-after-read window vs
                conv3 of image n-2 to a single 468ns op). Merge: 3 DVE
                tensor_tensor adds into q0, 5 gpsimd DMA-accumulates
                (single-DMA per add: inner run 1568 elems < 2048 so no
                descriptor splitting corruption).
                """
                cs, P = CTILES[m]
                s = slot(n, m)
                o1r = o1[(m, s)].rearrange("p (r c) -> p r c", c=WP)
                acc = o2[(m, s)][:, h * HROWS * W : (h + 1) * HROWS * W]
                PP = 128 if m == 4 else P
                two_act = (n, m) in ACT2_TILES

                def tap(k):
                    ky, kx = divmod(k, 3)
                    return o1r[
                        0:PP, h * HROWS + ky : h * HROWS + ky + HROWS, kx : kx + 56
                    ]

                wd1 = lambda k: wds[0:PP, m * 9 + k : m * 9 + k + 1]
                HP = HROWS * W

                qt = {}

                def q(k, tag, bufs):
                    qt[k] = scrp.tile([128, HP], F16, name=tag, tag=tag, bufs=bufs)
                    return qt[k]

                # products (back-to-back to free the o1 slot early)
                nc.vector.tensor_scalar(
                    q(0, "qe", 4)[0:PP, :], tap(0), wd1(0), b2c[0:PP, m : m + 1],
                    AOP.mult, AOP.add,
                )
                for k in (1, 2, 3):   # DVE -> TT-added into q0
                    nc.vector.tensor_scalar(
                        q(k, "qa", 2)[0:PP, :], tap(k), wd1(k), None, AOP.mult
                    )
                tt_tile = (n, m) in TT_TILES
                for k in (4, 5, 6):   # DVE -> DMA-merged (or TT for TT_TILES)
                    nc.vector.tensor_scalar(
                        q(k, "qb", 5)[0:PP, :], tap(k), wd1(k), None, AOP.mult
                    )
                # ACT products (tap 7 unless DVE7; tap 8 if two_act else DVE)
                if (n, m) in DVE7_TILES:
                    nc.vector.tensor_scalar(
                        q(7, "qc", 2)[0:PP, :], tap(7), wd1(7), None, AOP.mult
                    )
                else:
                    nc.scalar.activation(
                        q(7, "qc", 2)[0:PP, :], tap(7), AF.Copy, scale=wd1(7)
                    )
                if two_act:
                    nc.scalar.activation(
                        q(8, "qd", 3)[0:PP, :], tap(8), AF.Copy, scale=wd1(8)
                    )
                else:
                    nc.vector.tensor_scalar(
                        q(8, "qd", 3)[0:PP, :], tap(8), wd1(8), None, AOP.mult
                    )
                # DVE adds into q0
                for k in (1, 2, 3):
                    nc.vector.tensor_add(
                        qt[0][0:PP, :], qt[0][0:PP, :], qt[k][0:PP, :]
                    )

                if tt_tile:
                    for k in (4, 5, 6, 7, 8):
                        nc.vector.tensor_add(
                            qt[0][0:PP, :], qt[0][0:PP, :], qt[k][0:PP, :]
                        )
                else:
                    def dma_acc(dst, src):
                        nc.gpsimd.dma_start(
                            dst[0:PP, :], src[0:PP, :], accum_op=AOP.add
                        )

                    # DMA-accum tree; 0+=8 runs early (parallel to the q4
                    # side); q0 takes 3 serial accums but q4/q6 free earlier.
                    dma_acc(qt[4], qt[5])
                    dma_acc(qt[6], qt[7])
                    dma_acc(qt[0], qt[8])
                    dma_acc(qt[0], qt[4])
                    dma_acc(qt[0], qt[6])
                # relu into o2 (the only o2 write)
                nc.vector.tensor_scalar_max(acc[0:PP, :], qt[0][0:PP, :], 0.0)

            def dw_g(n, m):
                dw_g_half(n, m, 0)
                dw_g_half(n, m, 1)

            def dw(n):
                """emit dw tiles for image n; pair tile alongside even images
                (its conv1 runs right after the odd image's)."""
                keys = [(n, m) for m in range(4)]
                if n % 2 == 0:
                    keys.append((n, 4))
                # DVE-route first: their products/DMA tails overlap PE matmuls
                for key in keys:
                    if key not in PE_TILES:
                        dw_g(*key)
                for key in keys:
                    if key in PE_TILES:
                        dw_pe(*key)

            def conv3(n):
                """1x1 project + bias, residual via eye96 PSUM preload;
                eviction + bias on Pool."""
                half = 0 if n % 2 == 0 else 64
                for j in range(NCH):
                    ps = ps3p.tile([128, CHUNK], F32, name="ps3", tag="ps3")
                    nc.tensor.matmul(
                        ps[0:C, :],
                        eye96[:, :],
                        x16[n][:, j * CHUNK : (j + 1) * CHUNK],
                        start=True, stop=False,
                    )
                    for kt in range(4):
                        nc.tensor.matmul(
                            ps[0:C, :],
                            w3t[0:128, kt * C : kt * C + C],
                            o2[(kt, n % 2)][0:128, j * CHUNK : (j + 1) * CHUNK],
                            start=False, stop=False,
                        )
                    nc.tensor.matmul(
                        ps[0:C, :],
                        w3t[half : half + 64, 4 * C : 4 * C + C],
                        o2[(4, (n // 2) % 2)][
                            half : half + 64, j * CHUNK : (j + 1) * CHUNK
                        ],
                        start=False, stop=True,
                    )
                    o3 = yp.tile([C, CHUNK], F16, name="o3", tag="o3")
                    # GPSIMD cannot read PSUM on real HW; DVE early (its
                    # mid-pipeline dip), ACT for the tail images (ACT drains
                    # first).
                    if n < 2:
                        nc.vector.tensor_scalar(
                            o3[:], ps[0:C, :], b3c[:], None, AOP.add
                        )
                    else:
                        nc.scalar.activation(
                            o3[:], ps[0:C, :], AF.Identity, bias=b3c[:]
                        )
                    nc.sync.dma_start(
                        y_d[n, :, j * CHUNK : (j + 1) * CHUNK], o3[:]
                    )

            # ---- pipeline ----------------------------------------------------
            # G-routed ctiles first per image: their evicts unblock DVE early
            def c1_order(n):
                ms = list(range(4))
                return sorted(ms, key=lambda m: (n, m) in PE_TILES)

            for ma, mb in zip(c1_order(0), c1_order(1)):
                conv1(0, ma)
                conv1(1, mb)
            conv1(0, 4)
            dw(0)
            for m in c1_order(2):
                conv1(2, m)
            dw(1)
            conv3(0)
            load_x16(3)
            for m in c1_order(3):
                conv1(3, m)
            conv1(2, 4)
            dw(2)           # PE: (2,0),(2,1); G: (2,2),(2,3) + pair(2,4)
            conv3(1)
            dw(3)           # PE: (3,0),(3,1),(3,2); G: (3,3)
            conv3(2)
            conv3(3)

    nc.compile()
    return nc


def _fold_bn(inputs):
    """fold BN params into conv weights/biases; build device-side arrays"""
    f = lambda k: np.asarray(inputs[k], np.float32)
    w1, g1, b1, m1, v1 = f("w1"), f("g1"), f("b1"), f("m1"), f("v1")
    wd, g2, b2, m2, v2 = f("wd"), f("g2"), f("b2"), f("m2"), f("v2")
    w3, g3, b3, m3, v3 = f("w3"), f("g3"), f("b3"), f("m3"), f("v3")

    s1 = g1 / np.sqrt(v1 + EPS)
    W1p = w1[:, :, 0, 0] * s1[:, None]              # [M, C]
    b1p = b1 - m1 * s1                              # [M]
    s2 = g2 / np.sqrt(v2 + EPS)
    wdp = wd[:, 0] * s2[:, None, None]              # [M, 3, 3]
    b2p = b2 - m2 * s2
    s3 = g3 / np.sqrt(v3 + EPS)
    W3p = w3[:, :, 0, 0] * s3[:, None]              # [C, M]
    b3p = b3 - m3 * s3

    w1t = np.ascontiguousarray(W1p.T).astype(np.float16)  # [C, M]

    dgm = np.zeros((128, len(PE_DG_CTILES) * 9 * 128), np.float32)
    wds = np.zeros((128, NCT * 9), np.float32)
    for m, (cs, P) in enumerate(CTILES):
        for k in range(9):
            ky, kx = divmod(k, 3)
            w = wdp[cs : cs + P, ky, kx]
            if m == 4:
                wds[0:64, m * 9 + k] = w
                wds[64:128, m * 9 + k] = w
            else:
                wds[:P, m * 9 + k] = w
            if m in PE_DG_CTILES:
                di = PE_DG_CTILES.index(m)
                if m == 4:
                    blk = dgm[:128, (di * 9 + k) * 128 : (di * 9 + k) * 128 + 128]
                    np.fill_diagonal(blk, np.concatenate([w, w]))
                else:
                    blk = dgm[:P, (di * 9 + k) * 128 : (di * 9 + k) * 128 + P]
                    np.fill_diagonal(blk, w)
    dgm = dgm.astype(np.float16)

    w3t = np.zeros((128, NCT * C), np.float32)
    for kt, (ks, K) in enumerate(CTILES):
        w3t[:K, kt * C : kt * C + C] = W3p.T[ks : ks + K, :]
        if kt == 4:
            w3t[64:128, kt * C : kt * C + C] = W3p.T[ks : ks + K, :]
    w3t = w3t.astype(np.float16)

    b1c = np.zeros((128, NCT), np.float32)
    b2c = np.zeros((128, NCT), np.float32)
    for m, (cs, P) in enumerate(CTILES):
        b1c[:P, m] = b1p[cs : cs + P]
        b2c[:P, m] = b2p[cs : cs + P]
        if m == 4:
            b1c[64:128, m] = b1p[cs : cs + P]
            b2c[64:128, m] = b2p[cs : cs + P]
    b3c = b3p.reshape(C, 1).astype(np.float32)
    eye = np.eye(C, dtype=np.float32).astype(np.float16)

    return dict(
        w1t=w1t, dg=dgm, wds=wds, w3t=w3t, b1c=b1c, b2c=b2c, b3c=b3c, eye96=eye
    )


def _in_maps(inputs):
    params = _fold_bn(inputs)
    x = np.asarray(inputs["x"], np.float32)
    xr = x.reshape(N_CORES, NIMG, C, PIX)
    return [
        dict(
            x16=np.ascontiguousarray(xr[c]).astype(np.float16),
            **params,
        )
        for c in range(N_CORES)
    ]


def kernel(**inputs):
    if "nc" not in _CACHE:
        _CACHE["nc"] = _build()
    nc = _CACHE["nc"]

    B = np.asarray(inputs["x"]).shape[0]
    in_maps = _in_maps(inputs)
    res = run_bass_kernel_spmd(nc, in_maps, core_ids=list(range(N_CORES)))
    out = np.stack(
        [np.asarray(res.results[c]["y"]).astype(np.float32) for c in range(N_CORES)]
    )
    return out.reshape(B, C, H, W)


if __name__ == "__main__":
    rng = np.random.default_rng(0)
    inputs = dict(
        x=rng.standard_normal((32, C, H, W), dtype=np.float32),
        w1=(rng.standard_normal((M, C, 1, 1)) * 0.05).astype(np.float32),
        g1=np.ones(M, np.float32), b1=np.zeros(M, np.float32),
        m1=(rng.standard_normal(M) * 0.1).astype(np.float32),
        v1=rng.uniform(0.5, 1.5, M).astype(np.float32),
        wd=(rng.standard_normal((M, 1, 3, 3)) * 0.1).astype(np.float32),
        g2=np.ones(M, np.float32), b2=np.zeros(M, np.float32),
        m2=(rng.standard_normal(M) * 0.1).astype(np.float32),
        v2=rng.uniform(0.5, 1.5, M).astype(np.float32),
        w3=(rng.standard_normal((C, M, 1, 1)) * 0.05).astype(np.float32),
        g3=np.ones(C, np.float32), b3=np.zeros(C, np.float32),
        m3=np.zeros(C, np.float32), v3=np.ones(C, np.float32),
    )
    out = kernel(**inputs)
    print("kernel out", out.shape, out.dtype)
